# revision 1
# baseline (speedup 1.0000x reference)
"""Trainium2 Bass kernel for nn_DensityDecoder (gnn_message_passing).

Math: for every ordered pair (i, j) of NB=640 orbitals,
    pair = orb_i + orb_j
    qn   = LayerNorm(pair) ; q = qn @ Wq + bq
    attn = softmax(q . k / sqrt(Dh)) over a tiny T=32 latent KV
    out  = MLP(attn @ V @ Wo)  ->  2 values -> rho[i, j] = out0 + 1j*out1

LN statistics decompose exactly over pair = orb_i + orb_j, so the whole
pre-softmax pipeline collapses to per-orbital precomputes projected into
(head, token) score space:
    scores_ij = rstd_ij * (SA_i + SA_j - mu_ij*Sw) + Sb      (pre-scaled 1/sqrt(Dh))

Key restructure vs the previous version: rstd is folded INTO the matmul
operands (lhsT columns carry [rstd; rstd*mu; 1], and SA_j arrives via an
identity matmul against a per-row rstd-scaled copy of SA), so the scores land
in PSUM fully scaled and the Exp activation needs no per-tile scale vector.
That merges Exps across tiles, removes the per-row stats DMA round-trip
(all per-row operands live in SBUF, packed 4 row-groups per partition-block
at base partitions 0/32/64/96), and the score+chain pipeline runs in fp16
(validated 1.0e-3 rel err vs the fp64 oracle; budget 2e-2).

rho is symmetric; only j-blocks >= i-block are computed (240 of 400 tiles),
the lower triangle is mirrored host-side.

Sharding: rows i striped across 8 cores (i % 8 == core): identical SPMD
instruction stream, 80 rows -> 240 tiles of 128 pairs -> 60 chunks.
"""

import os
import numpy as np

EPS = 1e-5
H = 8
D = 256
T = 32
Dh = D // H
NB = 640
NCORES = 8
NBLK = NB // 128          # 5 column blocks
RPB = 128 // NCORES       # 16 rows per block per core
NROWS = NBLK * RPB        # 80 rows per core
TILES = [(B, r, jt) for B in range(NBLK) for r in range(RPB) for jt in range(B, NBLK)]
NTILES = len(TILES)       # 240
CHUNK = 4                 # tiles per chain chunk (512 pair columns)
GROUP = int(os.environ.get("DD_GROUP", "8"))  # chunks per superchunk
NCHUNKS = NTILES // CHUNK  # 60

_CACHE = {}


def _silu(x):
    return x / (1.0 + np.exp(-x))


def _ln(x, g, b):
    mu = x.mean(-1, keepdims=True)
    var = x.var(-1, keepdims=True)
    return (x - mu) / np.sqrt(var + EPS) * g + b


def _precompute(inputs):
    """Pair-independent precompute (all O(NB*D) or smaller)."""
    f = {}
    for k, v in inputs.items():
        v = np.asarray(v)
        f[k] = v.astype(np.float64) if v.dtype in (np.float32, np.float64) else v
    Z = np.asarray(inputs["Z"]).astype(np.int64)
    l = np.asarray(inputs["l"]).astype(np.int64)
    m = np.asarray(inputs["m"]).astype(np.int64)
    m_idx = np.clip(m + 3, 0, 4)
    emb = np.concatenate([f["elem_tab"][Z], f["l_tab"][l], f["m_tab"][m_idx]], -1)
    orb = _silu(emb @ f["Wp0"] + f["bp0"]) @ f["Wp1"] + f["bp1"]          # (NB, D)

    kv = _ln(f["latent"], f["ln_gkv"], f["ln_bkv"])
    k = (kv @ f["Wk"] + f["bk"]).reshape(T, H, Dh)
    v = (kv @ f["Wv"] + f["bv"]).reshape(T, H, Dh)

    g, b = f["ln_gq"], f["ln_bq"]
    mu = orb.mean(-1)
    msq = (orb * orb).mean(-1)

    A = (orb * g) @ f["Wq"]
    wbar = g @ f["Wq"]
    bq_eff = b @ f["Wq"] + f["bqa"]

    kT = k.transpose(1, 2, 0)                                            # (H, Dh, T)
    scale = 1.0 / np.sqrt(np.float64(Dh))

    def to_scores(x):
        xh = x.reshape(x.shape[:-1] + (H, Dh))
        return (np.einsum('...hd,hdt->...ht', xh, kT).reshape(x.shape[:-1] + (H * T,))
                * scale)

    SA = to_scores(A)                                                    # (NB, 256)
    Sw = to_scores(wbar)                                                 # (256,)
    Sb = to_scores(bq_eff)                                               # (256,)
    Wvo = np.einsum('thd,hde->hte', v, f["Wo"].reshape(H, Dh, D)).reshape(H * T, D)
    # fuse consecutive linear layers (no nonlinearity between them)
    Wa = Wvo @ f["Wt0"]
    ba = f["bo"] @ f["Wt0"] + f["bt0"]
    Wb = f["Wt1"] @ f["Wd0"]
    bb = f["bt1"] @ f["Wd0"] + f["bd0"]

    fl = lambda x: np.ascontiguousarray(x, np.float32)
    return {
        "SA": fl(SA), "Sw": fl(Sw), "Sb": fl(Sb), "mu": fl(mu), "msq": fl(msq),
        "orbT_s": fl(orb.T * np.sqrt(2.0 / D)),                          # (D, NB)
        "Wa": fl(Wa), "ba": fl(ba), "Wb": fl(Wb), "bb": fl(bb),
        "Wd1": fl(f["Wd1"]), "bd1": fl(f["bd1"]),
        "Wd2": fl(f["Wd2"]), "bd2": fl(f["bd2"]),
    }


def core_rows(c):
    return [B * 128 + r * NCORES + c for B in range(NBLK) for r in range(RPB)]


# by-need l3slab column offsets: rows of block B only use j >= B*128
L3_LEN = [NB - 128 * B for B in range(NBLK)]            # 640,512,384,256,128
L3_OFF = [RPB * sum(L3_LEN[:B]) for B in range(NBLK)]   # block base offsets
L3_TOT = RPB * sum(L3_LEN)                              # 30720


def l3_col(r_loc, jt):
    B, rr = r_loc // RPB, r_loc % RPB
    return L3_OFF[B] + rr * L3_LEN[B] + (jt - B) * 128


def _core_inputs(pc, c):
    rows = core_rows(c)
    f16 = np.float16
    # per local row r the rank-3 rhs rows [SA_i; -Sw; Sb]
    r3rows = np.zeros((NROWS, 3, 256), f16)
    for r, i in enumerate(rows):
        r3rows[r, 0] = pc["SA"][i]
        r3rows[r, 1] = -pc["Sw"]
        r3rows[r, 2] = pc["Sb"]
    ones80 = np.ones(NROWS, np.float32)
    return {
        "sa_in": pc["SA"].astype(f16),
        "r3rows_in": r3rows,
        "orbT_in": pc["orbT_s"],
        "orbTc_in": np.ascontiguousarray(pc["orbT_s"][:, rows]),
        "lhs_mu": np.ascontiguousarray(np.stack([ones80, pc["mu"][rows]])),
        "lhs_msq": np.ascontiguousarray(np.stack([ones80, pc["msq"][rows]])),
        "rhs_mu": np.ascontiguousarray(np.stack([pc["mu"], np.ones(NB, np.float32)])),
        "rhs_msq": np.ascontiguousarray(np.stack([pc["msq"], np.ones(NB, np.float32)])),
        "ident_in": np.eye(128, dtype=np.float32),
        "ident16_in": np.eye(128, dtype=f16),
        "wa": pc["Wa"], "wb": pc["Wb"],
        "wd1": pc["Wd1"], "wd2": pc["Wd2"],
        "ba_in": pc["ba"], "bb_in": pc["bb"], "bd1_in": pc["bd1"],
    }


def _build_nc(n_chunks):
    import concourse.bass as bass
    import concourse.bacc as bacc
    import concourse.tile as tile
    from concourse import mybir
    dt = mybir.dt
    f32 = dt.float32
    f32r = dt.float32r
    f16 = dt.float16
    AF = mybir.ActivationFunctionType
    AX = mybir.AxisListType

    nc = bacc.Bacc(None, target_bir_lowering=False)

    ein = lambda name, shape, d=f32: nc.dram_tensor(name, shape, d,
                                                     kind="ExternalInput")
    sa_in = ein("sa_in", [NB, 256], f16)
    r3rows_in = ein("r3rows_in", [NROWS, 3, 256], f16)
    orbT_in = ein("orbT_in", [D, NB], f32r)
    orbTc_in = ein("orbTc_in", [D, NROWS], f32r)
    lhs_mu = ein("lhs_mu", [2, NROWS], f32r)
    lhs_msq = ein("lhs_msq", [2, NROWS], f32r)
    rhs_mu = ein("rhs_mu", [2, NB], f32r)
    rhs_msq = ein("rhs_msq", [2, NB], f32r)
    ident_in = ein("ident_in", [128, 128], f32r)
    ident16_in = ein("ident16_in", [128, 128], f16)
    wa = ein("wa", [256, 256], f32r)
    wb = ein("wb", [256, 256], f32r)
    wd1 = ein("wd1", [256, 256], f32r)
    wd2 = ein("wd2", [256, 2], f32r)
    ba_in = ein("ba_in", [256])
    bb_in = ein("bb_in", [256])
    bd1_in = ein("bd1_in", [256])

    out_ext = nc.dram_tensor("out", [NCHUNKS, 2, 512], f32, kind="ExternalOutput")
    # combined per-row operands: [rstd | SA_i; rstd*mu | -Sw; ones | Sb]
    row_scratch = nc.dram_tensor("row_scratch", [3, NROWS, NB + 256], f16)

    with tile.TileContext(nc) as tc, \
            nc.allow_low_precision(reason="fp16 pipeline by design"):
        with (
            tc.tile_pool(name="const", bufs=1) as const,
            tc.tile_pool(name="ssa", bufs=int(os.environ.get("DD_SSA", "3"))) as ssa_pool,
            tc.tile_pool(name="prow", bufs=4) as prow,
            tc.tile_pool(name="score", bufs=int(os.environ.get("DD_SCORE", "3"))) as score,
            tc.tile_pool(name="attnp", bufs=2 * GROUP + 2) as attn_pool,
            tc.tile_pool(name="small", bufs=2) as small,
            tc.tile_pool(name="outp", bufs=2) as outp,
        ):
            # ---- constants into SBUF ----
            sa16 = const.tile([128, NBLK, 256], f16)
            nc.sync.dma_start(out=sa16, in_=sa_in.rearrange("(jt p) c -> p jt c", p=128))
            orbT = const.tile([128, 2, NB], f32r)
            nc.sync.dma_start(out=orbT, in_=orbT_in.rearrange("(k p) n -> p k n", p=128))
            orbTc = const.tile([128, 2, NROWS], f32r)
            nc.sync.dma_start(out=orbTc, in_=orbTc_in.rearrange("(k p) m -> p k m", p=128))
            lmu = const.tile([2, NROWS], f32r)
            nc.sync.dma_start(out=lmu, in_=lhs_mu[:])
            lmsq = const.tile([2, NROWS], f32r)
            nc.sync.dma_start(out=lmsq, in_=lhs_msq[:])
            rmu = const.tile([2, NB], f32r)
            nc.sync.dma_start(out=rmu, in_=rhs_mu[:])
            rmsq = const.tile([2, NB], f32r)
            nc.sync.dma_start(out=rmsq, in_=rhs_msq[:])

            w_a = const.tile([128, 2, 256], f32r)
            nc.sync.dma_start(out=w_a, in_=wa.rearrange("(k p) n -> p k n", p=128))
            w_b = const.tile([128, 2, 256], f32r)
            nc.sync.dma_start(out=w_b, in_=wb.rearrange("(k p) n -> p k n", p=128))
            w_d1 = const.tile([128, 2, 256], f32r)
            nc.sync.dma_start(out=w_d1, in_=wd1.rearrange("(k p) n -> p k n", p=128))
            w_d2 = const.tile([128, 2, 2], f32r)
            nc.sync.dma_start(out=w_d2, in_=wd2.rearrange("(k p) n -> p k n", p=128))

            b_a = const.tile([128, 2], f32)
            nc.sync.dma_start(out=b_a, in_=ba_in.rearrange("(m p) -> p m", p=128))
            b_b = const.tile([128, 2], f32)
            nc.sync.dma_start(out=b_b, in_=bb_in.rearrange("(m p) -> p m", p=128))
            b_d1 = const.tile([128, 2], f32)
            nc.sync.dma_start(out=b_d1, in_=bd1_in.rearrange("(m p) -> p m", p=128))

            ident = const.tile([128, 128], f32r)
            nc.sync.dma_start(out=ident, in_=ident_in[:])
            ident16 = const.tile([128, 128], f16)
            nc.sync.dma_start(out=ident16, in_=ident16_in[:])
            eps_t = const.tile([NROWS, 1], f32)
            nc.vector.memset(eps_t, EPS)

            # l3slab: per local row the rank-3 lhsT rows [rstd; rstd*mu_p; 1]
            # over all 640 j's, packed like r3slab (4 groups x 20 rows)
            l3slab = const.tile([3, L3_TOT], f16)
            rstd_T = const.tile([128, NBLK, NROWS], f16)

            # ---- prologue: per-pair LN stats for this core's 80 rows ----
            with (
                tc.tile_pool(name="pro_ps", bufs=2, space="PSUM") as pro_ps,
                tc.tile_pool(name="pro_sb", bufs=1) as pro_sb,
            ):
                mu_p_sb = pro_sb.tile([NROWS, NB], f32r, tag="mu_p")
                rstd_sb = pro_sb.tile([NROWS, NB], f32r, tag="rstd")
                invr_sb = pro_sb.tile([NROWS, NB], f32r, tag="invr")
                for nch in range(2):
                    seg = slice(nch * 320, (nch + 1) * 320)
                    psA = pro_ps.tile([NROWS, 320], f32, tag="psA")
                    nc.tensor.matmul(psA, lmu, rmu[:, seg], start=True, stop=True)
                    nc.vector.tensor_copy(out=mu_p_sb[:, seg], in_=psA)
                    psB = pro_ps.tile([NROWS, 320], f32, tag="psB")
                    nc.tensor.matmul(psB, lmsq, rmsq[:, seg], start=True, stop=False)
                    nc.tensor.matmul(psB, orbTc[:, 0, :], orbT[:, 0, seg],
                                     start=False, stop=False)
                    nc.tensor.matmul(psB, orbTc[:, 1, :], orbT[:, 1, seg],
                                     start=False, stop=True)
                    mu2 = pro_sb.tile([NROWS, 320], f32, tag="mu2")
                    nc.vector.tensor_mul(mu2, mu_p_sb[:, seg], mu_p_sb[:, seg])
                    nc.vector.tensor_sub(invr_sb[:, seg], psB, mu2)
                # invr = sqrt(var + eps); rstd = 1/invr
                nc.scalar.activation(out=invr_sb, in_=invr_sb, func=AF.Sqrt,
                                     bias=eps_t[:, 0:1])
                nc.vector.reciprocal(out=rstd_sb, in_=invr_sb)
                # rstd*mu_p, and fp16 casts of both rows
                rstdmu = pro_sb.tile([NROWS, NB], f32, tag="rstdmu")
                nc.vector.tensor_mul(rstdmu, rstd_sb, mu_p_sb)
                rstd16 = pro_sb.tile([NROWS, NB], f16, tag="rstd16")
                nc.vector.tensor_copy(out=rstd16, in_=rstd_sb)
                rstdmu16 = pro_sb.tile([NROWS, NB], f16, tag="rstdmu16")
                nc.vector.tensor_copy(out=rstdmu16, in_=rstdmu)
                # assemble the combined per-row operand planes in DRAM
                nc.sync.dma_start(out=row_scratch[0, :, 0:NB], in_=rstd16)
                nc.sync.dma_start(out=row_scratch[1, :, 0:NB], in_=rstdmu16)
                ones16 = pro_sb.tile([NROWS, NB], f16, tag="ones16")
                nc.vector.memset(ones16, 1.0)
                nc.sync.dma_start(out=row_scratch[2, :, 0:NB], in_=ones16)
                nc.sync.dma_start(out=row_scratch[:, :, NB:],
                                  in_=r3rows_in.rearrange("q k n -> k q n"))
                # transposed rstd for the per-row scaled-SA products
                for jt in range(NBLK):
                    pT = pro_ps.tile([128, NROWS], f32r, tag="pT")
                    nc.tensor.transpose(
                        pT, rstd_sb[:, jt * 128:(jt + 1) * 128],
                        ident[0:NROWS, 0:NROWS])
                    nc.vector.tensor_copy(out=rstd_T[:, jt, :], in_=pT)

            # ---- main loop ----
            from concourse.tile_rust import add_dep_helper
            import contextlib
            _mstack = contextlib.ExitStack()
            attnT_pool = _mstack.enter_context(
                tc.tile_pool(name="attnT", bufs=GROUP + 2))
            chainx = _mstack.enter_context(
                tc.tile_pool(name="chainx", bufs=int(os.environ.get("DD_CHX", "3"))))
            px3_pool = _mstack.enter_context(
                tc.tile_pool(name="px3", bufs=int(os.environ.get("DD_PX3", "2")), space="PSUM"))
            ptr_pool = _mstack.enter_context(
                tc.tile_pool(name="ptr", bufs=int(os.environ.get("DD_PTR", "2")), space="PSUM"))
            pchain = _mstack.enter_context(
                tc.tile_pool(name="pchain", bufs=int(os.environ.get("DD_PCH", "2")), space="PSUM"))
            repeat = int(os.environ.get("DD_REPEAT", "1"))
            attn16 = bool(int(os.environ.get("DD_ATTN16", "1")))

            act_prev = [None]
            nopin = bool(int(os.environ.get("DD_NOPIN", "0")))

            def act_chain(bi):
                if act_prev[0] is not None and not nopin:
                    add_dep_helper(bi.ins, act_prev[0].ins, sync=True,
                                   reason="pin ACT order for act-table reuse")
                act_prev[0] = bi
                return bi

            row_stage = {}            # r_loc -> (ssa tile, r3 tile)

            def stage_row(r_loc):
                if r_loc in row_stage:
                    return row_stage[r_loc]
                # rstd-scaled SA: ssa[p, jt, s] = rstd[i, jt*128+p] * SA[jt*128+p, s]
                ssa = ssa_pool.tile([128, NBLK, 256], f16, tag="ssa", name="ssa")
                B = r_loc // RPB  # this row's block: only jt >= B is used
                nc.vector.tensor_mul(
                    ssa[:, B:, :], sa16[:, B:, :],
                    rstd_T[:, B:, r_loc:r_loc + 1].to_broadcast([128, NBLK - B, 256]))
                rb = prow.tile([3, NB + 256], f16, tag="rb", name="rb")
                nc.sync.dma_start(out=rb, in_=row_scratch[:, r_loc, :])
                row_stage[r_loc] = (ssa, rb)
                return row_stage[r_loc]

            def ensure_row(r_loc):
                res = stage_row(r_loc)
                for ahead in (1, 2):       # prefetch upcoming rows
                    if r_loc + ahead < NROWS:
                        stage_row(r_loc + ahead)
                return res

            def score_softmax(t0):
                # two consecutive tiles share one psum bank; scores arrive
                # fully rstd-scaled so a single unscaled Exp covers both
                px3 = px3_pool.tile([128, 2, 256], f32, tag="px3", name="px3")
                metas = []
                for ti in range(2):
                    B, r, jt = TILES[t0 + ti]
                    r_loc = B * RPB + r
                    ssa, rb = ensure_row(r_loc)
                    metas.append((r_loc, jt, ssa))
                    lhsT = rb[:, jt * 128:jt * 128 + 128]
                    # the first matmul into the bank starts the accumulation
                    # group (start zeroes the whole 2KB zero region)
                    nc.tensor.matmul(px3[:, ti, :], lhsT, rb[:, NB:],
                                     start=(ti == 0), stop=False)
                if metas[0][0] == metas[1][0] and metas[1][1] == metas[0][1] + 1:
                    r_loc, jt0, ssa = metas[0]
                    nc.tensor.matmul(
                        px3.rearrange("p a s -> p (a s)"), ident16,
                        ssa[:, jt0:jt0 + 2, :].rearrange("p a s -> p (a s)"),
                        start=False, stop=True)
                else:
                    for ti, (r_loc, jt, ssa) in enumerate(metas):
                        nc.tensor.matmul(px3[:, ti, :], ident16, ssa[:, jt, :],
                                         start=False, stop=(ti == 1))
                ee = score.tile([128, 2, 8, 32], f32, tag="ee", name="ee")
                act_chain(nc.scalar.activation(
                    out=ee.rearrange("p a h t -> p (a h t)"),
                    in_=px3.rearrange("p a s -> p (a s)"),
                    func=AF.Exp))
                return ee

            def transpose_chunk(attn_pair, ptrt):
                # 4 tiles (2 attn tiles) -> attnT [128s, kt, (c p)]
                tr_ident = ident16 if attn16 else ident
                for s2, attn in enumerate(attn_pair):
                    for ti in range(2):
                        s = s2 * 2 + ti
                        a2 = attn[:, ti, :, :].rearrange("p h t -> p (h t)")
                        sseg = slice(s * 128, (s + 1) * 128)
                        nc.tensor.transpose(ptrt[:, 0, sseg], a2[:, 0:128],
                                            tr_ident)
                        nc.tensor.transpose(ptrt[:, 1, sseg], a2[:, 128:256],
                                            tr_ident)

            def chain_pair(aT_pair, q_pair):
                # two chunks share each silu: one [128, 1024] activation per
                # (layer, mt) halves the scalar engine's fixed per-op cost.
                # Loop order (mt, kt, qi) keeps each weight tile loaded for
                # two consecutive matmuls.
                def layer(x_of, w, b_tile, out_tile):
                    for mt in range(2):
                        ps = pchain.tile([128, 2, 512], f32, tag="pch",
                                         name="pch")
                        for kt in range(2):
                            for qi in range(2):
                                nc.tensor.matmul(
                                    ps[:, qi, :],
                                    w[:, kt, mt * 128:(mt + 1) * 128],
                                    x_of(qi, kt),
                                    start=(kt == 0), stop=(kt == 1))
                        act_chain(nc.scalar.activation(
                            out=out_tile[:, mt, :, :].rearrange(
                                "p q n -> p (q n)"),
                            in_=ps.rearrange("p q n -> p (q n)"), func=AF.Silu,
                            bias=b_tile[:, mt:mt + 1]))

                x2 = chainx.tile([128, 2, 2, 512], f32r, tag="x", name="x2")
                layer(lambda qi, kt: aT_pair[qi][:, kt, :], w_a, b_a, x2)
                x4 = chainx.tile([128, 2, 2, 512], f32r, tag="x", name="x4")
                layer(lambda qi, kt: x2[:, kt, qi, :], w_b, b_b, x4)
                x5 = chainx.tile([128, 2, 2, 512], f32r, tag="x", name="x5")
                layer(lambda qi, kt: x4[:, kt, qi, :], w_d1, b_d1, x5)
                ps6 = pchain.tile([2, 2, 512], f32, tag="pch", name="ps6")
                for kt in range(2):
                    for qi in range(2):
                        nc.tensor.matmul(ps6[:, qi, :], w_d2[:, kt, :],
                                         x5[:, kt, qi, :],
                                         start=(kt == 0), stop=(kt == 1))
                # bias bd2 is added host-side during assembly
                o6 = outp.tile([2, 2, 512], f32, tag="o6", name="o6")
                nc.vector.tensor_copy(out=o6, in_=ps6)
                for qi in range(2):
                    nc.sync.dma_start(out=out_ext[q_pair[qi]], in_=o6[:, qi, :])

            n_super = (n_chunks + GROUP - 1) // GROUP

            def scores_phase(qs):
                # burst all scores: PE stays in back-to-back matmuls while
                # the softmax pipeline (Exp/reduce/recip/normalize) drains
                # behind on the other engines. One reduce per pair, one
                # reciprocal + two normalizes per two pairs.
                attns = []
                pend = []
                den4 = None
                for q in qs:
                    for s in range(0, CHUNK, 2):
                        ee = score_softmax(q * CHUNK + s)
                        if den4 is None:
                            den4 = small.tile([128, 4, 8], f32, tag="den",
                                              name="den")
                        h = len(pend)
                        nc.vector.reduce_sum(out=den4[:, 2 * h:2 * h + 2, :],
                                             in_=ee, axis=AX.X)
                        pend.append(ee)
                        if len(pend) == 2:
                            rden4 = small.tile([128, 4, 8], f32, tag="rden",
                                               name="rden")
                            nc.vector.reciprocal(out=rden4, in_=den4)
                            for h2, ee2 in enumerate(pend):
                                attn = attn_pool.tile(
                                    [128, 2, 8, 32], f16 if attn16 else f32r,
                                    tag="attn", name="attn")
                                nc.gpsimd.tensor_mul(
                                    attn, ee2,
                                    rden4[:, 2 * h2:2 * h2 + 2, :]
                                    .to_broadcast([128, 2, 8, 32]))
                                attns.append(attn)
                            pend = []
                            den4 = None
                return attns

            def transpose_phase(qs, attns):
                aTs = []
                for k, q in enumerate(qs):
                    ptrt = ptr_pool.tile([128, 2, 512], f16 if attn16 else f32r,
                                         tag="ptrt", name="ptrt")
                    transpose_chunk(attns[2 * k:2 * k + 2], ptrt)
                    aT = attnT_pool.tile([128, 2, 512], f32r, tag="aT",
                                         name="aT")
                    nc.vector.tensor_copy(out=aT, in_=ptrt)
                    aTs.append(aT)
                return aTs

            stage = int(os.environ.get("DD_STAGE", "9"))
            if stage < 2:
                # prologue-only bring-up: fill the output with a recognizable
                # constant so bass2jax has something to fetch
                dummy = outp.tile([2, 512], f32, tag="o6", name="dummy")
                nc.vector.memset(dummy, 0.5)
                for q in range(n_chunks):
                    nc.sync.dma_start(out=out_ext[q], in_=dummy)
            elif stage < 3:
                # scores-only bring-up
                for sc in range(n_super):
                    qs = list(range(sc * GROUP, min((sc + 1) * GROUP, n_chunks)))
                    transpose_phase(qs, scores_phase(qs))
                dummy = outp.tile([2, 512], f32, tag="o6", name="dummy")
                nc.vector.memset(dummy, 0.5)
                for q in range(n_chunks):
                    nc.sync.dma_start(out=out_ext[q], in_=dummy)
            else:
                for rep in range(repeat):
                    row_stage.clear()

                    def run_chains(p):
                        qs, aTs = p
                        for i in range(0, len(qs) - 1, 2):
                            chain_pair(aTs[i:i + 2], qs[i:i + 2])

                    def transpose_one(q, attn_pair, aTs):
                        ptrt = ptr_pool.tile(
                            [128, 2, 512], f16 if attn16 else f32r,
                            tag="ptrt", name="ptrt")
                        transpose_chunk(attn_pair, ptrt)
                        aT = attnT_pool.tile([128, 2, 512], f32r, tag="aT",
                                             name="aT")
                        nc.vector.tensor_copy(out=aT, in_=ptrt)
                        aTs.append(aT)

                    pending = None
                    for sc in range(n_super):
                        qs = list(range(sc * GROUP,
                                        min((sc + 1) * GROUP, n_chunks)))
                        attns = scores_phase(qs)
                        # interleave prev-super chain bursts with this
                        # super's transposes: the aT casts (DVE) hide under
                        # chain matmuls instead of stalling the PE
                        aTs = []
                        pq, paT = pending if pending else (None, None)
                        interleave = bool(int(os.environ.get("DD_ILV", "1")))
                        ci = 0
                        for k, q in enumerate(qs):
                            if (interleave and pending and k % 2 == 0
                                    and ci * 2 + 1 < len(pq)):
                                chain_pair(paT[ci * 2:ci * 2 + 2],
                                           pq[ci * 2:ci * 2 + 2])
                                ci += 1
                            transpose_one(q, attns[2 * k:2 * k + 2], aTs)
                        if pending:
                            # drain whatever chains of the previous super
                            # did not fit into the interleave slots
                            while ci * 2 + 1 < len(pq):
                                chain_pair(paT[ci * 2:ci * 2 + 2],
                                           pq[ci * 2:ci * 2 + 2])
                                ci += 1
                        pending = (qs, aTs)
                    if pending:
                        run_chains(pending)
            _mstack.close()
    nc.compile()
    return nc


def _get_nc(n_chunks):
    key = ("nc", n_chunks)
    if key not in _CACHE:
        _CACHE[key] = _build_nc(n_chunks)
    return _CACHE[key]


def kernel(**inputs):
    from concourse.bass_utils import run_bass_kernel_spmd

    n_chunks = int(os.environ.get("DD_CHUNKS", NCHUNKS))
    pc = _precompute(inputs)
    in_maps = [_core_inputs(pc, c) for c in range(NCORES)]
    nc = _get_nc(n_chunks)
    res = run_bass_kernel_spmd(nc, in_maps, core_ids=list(range(NCORES)),
                               trace=bool(int(os.environ.get("DD_TRACE", "0"))))
    _CACHE["last_result"] = res

    R = np.zeros((NB, NB, 2), np.float32)
    for c in range(NCORES):
        o = res.results[c]["out"] + pc["bd2"][None, :, None]   # (NCHUNKS, 2, 512)
        ot = o.reshape(NCHUNKS, 2, CHUNK, 128).transpose(0, 2, 1, 3).reshape(-1, 2, 128)
        for t in range(n_chunks * CHUNK):
            B, r, jt = TILES[t]
            i = B * 128 + r * NCORES + c
            R[i, jt * 128:(jt + 1) * 128, 0] = ot[t, 0]
            R[i, jt * 128:(jt + 1) * 128, 1] = ot[t, 1]
    for bi in range(NBLK):
        for bj in range(bi):
            R[bi * 128:(bi + 1) * 128, bj * 128:(bj + 1) * 128] = \
                R[bj * 128:(bj + 1) * 128, bi * 128:(bi + 1) * 128].transpose(1, 0, 2)

    rho = (R[:, :, 0] + 1j * R[:, :, 1]).astype(np.complex64)
    n_spin = int(np.asarray(inputs["n_spin"]))
    return np.broadcast_to(rho[None], (n_spin, NB, NB)).copy()



# revision 2
# speedup vs baseline: 1.2031x; 1.2031x over previous
"""Trainium2 Bass kernel for nn_DensityDecoder (gnn_message_passing).

Math: for every ordered pair (i, j) of NB=640 orbitals,
    pair = orb_i + orb_j
    qn   = LayerNorm(pair) ; q = qn @ Wq + bq
    attn = softmax(q . k / sqrt(Dh)) over a tiny T=32 latent KV
    out  = MLP(attn @ V @ Wo)  ->  2 values -> rho[i, j] = out0 + 1j*out1

LN statistics decompose exactly over pair = orb_i + orb_j, so the whole
pre-softmax pipeline collapses to per-orbital precomputes projected into
(head, token) score space:
    scores_ij = rstd_ij * (SA_i + SA_j - mu_ij*Sw) + Sb      (pre-scaled 1/sqrt(Dh))

Per 4-tile chunk the scores land in one [128, 4, 256] PSUM tile (rank-3
stats matmuls + rstd-scaled-SA identity adds), one Exp covers the chunk,
and the attn -> attn^T reshuffle for the feature chain runs on the DMA
XBAR transpose (16x128 tiles) instead of PE transpose matmuls, writing
fp16 straight into SBUF in [s, (tile,kt), pair] block layout.  The MLP
chain runs fp16 end to end (weights + activations; PSUM accumulation is
fp32).

rho is symmetric; only j-blocks >= i-block are computed (240 of 400 tiles),
the lower triangle is mirrored host-side.

Sharding: rows i striped across 8 cores (i % 8 == core): identical SPMD
instruction stream, 80 rows -> 240 tiles of 128 pairs -> 30 chain-pairs.
"""

import os
import numpy as np

EPS = 1e-5
H = 8
D = 256
T = 32
Dh = D // H
NB = 640
NCORES = 8
NBLK = NB // 128          # 5 column blocks
RPB = 128 // NCORES       # 16 rows per block per core
NROWS = NBLK * RPB        # 80 rows per core
TILES = [(B, r, jt) for B in range(NBLK) for r in range(RPB) for jt in range(B, NBLK)]
NTILES = len(TILES)       # 240
CHUNK = 4                 # tiles per score chunk (one [128, 4, 256] psum)
CPT = 2 * CHUNK           # tiles per chain-pair (2 chunks)
GROUP = int(os.environ.get("DD_GROUP", "8"))  # chunks per superchunk
NCHUNKS = NTILES // CHUNK  # 60
NCPS = NTILES // CPT       # 30

_CACHE = {}


def _silu(x):
    return x / (1.0 + np.exp(-x))


def _ln(x, g, b):
    mu = x.mean(-1, keepdims=True)
    var = x.var(-1, keepdims=True)
    return (x - mu) / np.sqrt(var + EPS) * g + b


def _precompute(inputs):
    """Pair-independent precompute (all O(NB*D) or smaller)."""
    f = {}
    for k, v in inputs.items():
        v = np.asarray(v)
        f[k] = v.astype(np.float64) if v.dtype in (np.float32, np.float64) else v
    Z = np.asarray(inputs["Z"]).astype(np.int64)
    l = np.asarray(inputs["l"]).astype(np.int64)
    m = np.asarray(inputs["m"]).astype(np.int64)
    m_idx = np.clip(m + 3, 0, 4)
    emb = np.concatenate([f["elem_tab"][Z], f["l_tab"][l], f["m_tab"][m_idx]], -1)
    orb = _silu(emb @ f["Wp0"] + f["bp0"]) @ f["Wp1"] + f["bp1"]          # (NB, D)

    kv = _ln(f["latent"], f["ln_gkv"], f["ln_bkv"])
    k = (kv @ f["Wk"] + f["bk"]).reshape(T, H, Dh)
    v = (kv @ f["Wv"] + f["bv"]).reshape(T, H, Dh)

    g, b = f["ln_gq"], f["ln_bq"]
    mu = orb.mean(-1)
    msq = (orb * orb).mean(-1)

    A = (orb * g) @ f["Wq"]
    wbar = g @ f["Wq"]
    bq_eff = b @ f["Wq"] + f["bqa"]

    kT = k.transpose(1, 2, 0)                                            # (H, Dh, T)
    scale = 1.0 / np.sqrt(np.float64(Dh))

    def to_scores(x):
        xh = x.reshape(x.shape[:-1] + (H, Dh))
        return (np.einsum('...hd,hdt->...ht', xh, kT).reshape(x.shape[:-1] + (H * T,))
                * scale)

    SA = to_scores(A)                                                    # (NB, 256)
    Sw = to_scores(wbar)                                                 # (256,)
    Sb = to_scores(bq_eff)                                               # (256,)
    Wvo = np.einsum('thd,hde->hte', v, f["Wo"].reshape(H, Dh, D)).reshape(H * T, D)
    # fuse consecutive linear layers (no nonlinearity between them)
    Wa = Wvo @ f["Wt0"]
    ba = f["bo"] @ f["Wt0"] + f["bt0"]
    Wb = f["Wt1"] @ f["Wd0"]
    bb = f["bt1"] @ f["Wd0"] + f["bd0"]

    fl = lambda x: np.ascontiguousarray(x, np.float32)
    return {
        "SA": fl(SA), "Sw": fl(Sw), "Sb": fl(Sb), "mu": fl(mu), "msq": fl(msq),
        "orbT_s": fl(orb.T * np.sqrt(2.0 / D)),                          # (D, NB)
        "Wa": fl(Wa), "ba": fl(ba), "Wb": fl(Wb), "bb": fl(bb),
        "Wd1": fl(f["Wd1"]), "bd1": fl(f["bd1"]),
        "Wd2": fl(f["Wd2"]), "bd2": fl(f["bd2"]),
    }


def core_rows(c):
    return [B * 128 + r * NCORES + c for B in range(NBLK) for r in range(RPB)]


def _core_inputs(pc, c):
    rows = core_rows(c)
    f16 = np.float16
    # per local row r the rank-3 rhs rows [SA_i; -Sw; Sb]
    r3rows = np.zeros((NROWS, 3, 256), f16)
    for r, i in enumerate(rows):
        r3rows[r, 0] = pc["SA"][i]
        r3rows[r, 1] = -pc["Sw"]
        r3rows[r, 2] = pc["Sb"]
    ones80 = np.ones(NROWS, np.float32)
    return {
        "sa_in": pc["SA"].astype(f16),
        "r3rows_in": r3rows,
        "orbT_in": pc["orbT_s"],
        "orbTc_in": np.ascontiguousarray(pc["orbT_s"][:, rows]),
        "lhs_mu": np.ascontiguousarray(np.stack([ones80, pc["mu"][rows]])),
        "lhs_msq": np.ascontiguousarray(np.stack([ones80, pc["msq"][rows]])),
        "rhs_mu": np.ascontiguousarray(np.stack([pc["mu"], np.ones(NB, np.float32)])),
        "rhs_msq": np.ascontiguousarray(np.stack([pc["msq"], np.ones(NB, np.float32)])),
        "ident_in": np.eye(128, dtype=np.float32),
        "ident16_in": np.eye(128, dtype=f16),
        "wa": pc["Wa"].astype(f16), "wb": pc["Wb"].astype(f16),
        "wd1": pc["Wd1"].astype(f16), "wd2": pc["Wd2"].astype(f16),
        "ba_in": pc["ba"], "bb_in": pc["bb"], "bd1_in": pc["bd1"],
    }


def _build_nc(n_chunks):
    import concourse.bass as bass
    import concourse.bacc as bacc
    import concourse.tile as tile
    from concourse import mybir
    dt = mybir.dt
    f32 = dt.float32
    f32r = dt.float32r
    f16 = dt.float16
    AF = mybir.ActivationFunctionType
    AX = mybir.AxisListType

    assert n_chunks % 2 == 0
    n_cps = n_chunks // 2

    nc = bacc.Bacc(None, target_bir_lowering=False)

    ein = lambda name, shape, d=f32: nc.dram_tensor(name, shape, d,
                                                     kind="ExternalInput")
    sa_in = ein("sa_in", [NB, 256], f16)
    r3rows_in = ein("r3rows_in", [NROWS, 3, 256], f16)
    orbT_in = ein("orbT_in", [D, NB], f32r)
    orbTc_in = ein("orbTc_in", [D, NROWS], f32r)
    lhs_mu = ein("lhs_mu", [2, NROWS], f32r)
    lhs_msq = ein("lhs_msq", [2, NROWS], f32r)
    rhs_mu = ein("rhs_mu", [2, NB], f32r)
    rhs_msq = ein("rhs_msq", [2, NB], f32r)
    ident_in = ein("ident_in", [128, 128], f32r)
    ident16_in = ein("ident16_in", [128, 128], f16)
    wa = ein("wa", [256, 256], f16)
    wb = ein("wb", [256, 256], f16)
    wd1 = ein("wd1", [256, 256], f16)
    wd2 = ein("wd2", [256, 2], f16)
    ba_in = ein("ba_in", [256])
    bb_in = ein("bb_in", [256])
    bd1_in = ein("bd1_in", [256])

    out_ext = nc.dram_tensor("out", [NCPS, 2, 1024], f32, kind="ExternalOutput")
    # combined per-row operands: [rstd | SA_i; rstd*mu | -Sw; ones | Sb]
    row_scratch = nc.dram_tensor("row_scratch", [3, NROWS, NB + 256], f16)

    with tile.TileContext(nc) as tc, \
            nc.allow_low_precision(reason="fp16 pipeline by design"):
        with (
            tc.tile_pool(name="const", bufs=1) as const,
            tc.tile_pool(name="ssa", bufs=int(os.environ.get("DD_SSA", "3"))) as ssa_pool,
            tc.tile_pool(name="prow", bufs=4) as prow,
            tc.tile_pool(name="ee", bufs=int(os.environ.get("DD_EE", "6"))) as ee_pool,
            tc.tile_pool(name="attnp", bufs=int(os.environ.get("DD_ATTN", "8"))) as attn_pool,
            tc.tile_pool(name="small", bufs=4) as small,
            tc.tile_pool(name="outp", bufs=2) as outp,
        ):
            # ---- constants into SBUF ----
            sa16 = const.tile([128, NBLK, 256], f16)
            nc.sync.dma_start(out=sa16, in_=sa_in.rearrange("(jt p) c -> p jt c", p=128))
            orbT = const.tile([128, 2, NB], f32r)
            nc.sync.dma_start(out=orbT, in_=orbT_in.rearrange("(k p) n -> p k n", p=128))
            orbTc = const.tile([128, 2, NROWS], f32r)
            nc.sync.dma_start(out=orbTc, in_=orbTc_in.rearrange("(k p) m -> p k m", p=128))
            lmu = const.tile([2, NROWS], f32r)
            nc.sync.dma_start(out=lmu, in_=lhs_mu[:])
            lmsq = const.tile([2, NROWS], f32r)
            nc.sync.dma_start(out=lmsq, in_=lhs_msq[:])
            rmu = const.tile([2, NB], f32r)
            nc.sync.dma_start(out=rmu, in_=rhs_mu[:])
            rmsq = const.tile([2, NB], f32r)
            nc.sync.dma_start(out=rmsq, in_=rhs_msq[:])

            w_a = const.tile([128, 2, 256], f16)
            nc.sync.dma_start(out=w_a, in_=wa.rearrange("(k p) n -> p k n", p=128))
            w_b = const.tile([128, 2, 256], f16)
            nc.sync.dma_start(out=w_b, in_=wb.rearrange("(k p) n -> p k n", p=128))
            w_d1 = const.tile([128, 2, 256], f16)
            nc.sync.dma_start(out=w_d1, in_=wd1.rearrange("(k p) n -> p k n", p=128))
            w_d2 = const.tile([128, 2, 2], f16)
            nc.sync.dma_start(out=w_d2, in_=wd2.rearrange("(k p) n -> p k n", p=128))

            b_a = const.tile([128, 2], f32)
            nc.sync.dma_start(out=b_a, in_=ba_in.rearrange("(m p) -> p m", p=128))
            b_b = const.tile([128, 2], f32)
            nc.sync.dma_start(out=b_b, in_=bb_in.rearrange("(m p) -> p m", p=128))
            b_d1 = const.tile([128, 2], f32)
            nc.sync.dma_start(out=b_d1, in_=bd1_in.rearrange("(m p) -> p m", p=128))

            ident = const.tile([128, 128], f32r)
            nc.sync.dma_start(out=ident, in_=ident_in[:])
            ident16 = const.tile([128, 128], f16)
            nc.sync.dma_start(out=ident16, in_=ident16_in[:])
            eps_t = const.tile([NROWS, 1], f32)
            nc.vector.memset(eps_t, EPS)

            rstd_T = const.tile([128, NBLK, NROWS], f16)

            # ---- prologue: per-pair LN stats for this core's 80 rows ----
            with (
                tc.tile_pool(name="pro_ps", bufs=2, space="PSUM") as pro_ps,
                tc.tile_pool(name="pro_sb", bufs=1) as pro_sb,
            ):
                mu_p_sb = pro_sb.tile([NROWS, NB], f32r, tag="mu_p")
                rstd_sb = pro_sb.tile([NROWS, NB], f32r, tag="rstd")
                invr_sb = pro_sb.tile([NROWS, NB], f32r, tag="invr")
                for nch in range(2):
                    seg = slice(nch * 320, (nch + 1) * 320)
                    psA = pro_ps.tile([NROWS, 320], f32, tag="psA")
                    nc.tensor.matmul(psA, lmu, rmu[:, seg], start=True, stop=True)
                    nc.vector.tensor_copy(out=mu_p_sb[:, seg], in_=psA)
                    psB = pro_ps.tile([NROWS, 320], f32, tag="psB")
                    nc.tensor.matmul(psB, lmsq, rmsq[:, seg], start=True, stop=False)
                    nc.tensor.matmul(psB, orbTc[:, 0, :], orbT[:, 0, seg],
                                     start=False, stop=False)
                    nc.tensor.matmul(psB, orbTc[:, 1, :], orbT[:, 1, seg],
                                     start=False, stop=True)
                    mu2 = pro_sb.tile([NROWS, 320], f32, tag="mu2")
                    nc.vector.tensor_mul(mu2, mu_p_sb[:, seg], mu_p_sb[:, seg])
                    nc.vector.tensor_sub(invr_sb[:, seg], psB, mu2)
                # invr = sqrt(var + eps); rstd = 1/invr
                nc.scalar.activation(out=invr_sb, in_=invr_sb, func=AF.Sqrt,
                                     bias=eps_t[:, 0:1])
                nc.vector.reciprocal(out=rstd_sb, in_=invr_sb)
                # rstd*mu_p, and fp16 casts of both rows
                rstdmu = pro_sb.tile([NROWS, NB], f32, tag="rstdmu")
                nc.vector.tensor_mul(rstdmu, rstd_sb, mu_p_sb)
                rstd16 = pro_sb.tile([NROWS, NB], f16, tag="rstd16")
                nc.vector.tensor_copy(out=rstd16, in_=rstd_sb)
                rstdmu16 = pro_sb.tile([NROWS, NB], f16, tag="rstdmu16")
                nc.vector.tensor_copy(out=rstdmu16, in_=rstdmu)
                # assemble the combined per-row operand planes in DRAM
                nc.sync.dma_start(out=row_scratch[0, :, 0:NB], in_=rstd16)
                nc.sync.dma_start(out=row_scratch[1, :, 0:NB], in_=rstdmu16)
                ones16 = pro_sb.tile([NROWS, NB], f16, tag="ones16")
                nc.vector.memset(ones16, 1.0)
                nc.sync.dma_start(out=row_scratch[2, :, 0:NB], in_=ones16)
                nc.sync.dma_start(out=row_scratch[:, :, NB:],
                                  in_=r3rows_in.rearrange("q k n -> k q n"))
                # transposed rstd for the per-row scaled-SA products
                for jt in range(NBLK):
                    pT = pro_ps.tile([128, NROWS], f32r, tag="pT")
                    nc.tensor.transpose(
                        pT, rstd_sb[:, jt * 128:(jt + 1) * 128],
                        ident[0:NROWS, 0:NROWS])
                    nc.vector.tensor_copy(out=rstd_T[:, jt, :], in_=pT)

            # ---- main loop ----
            import contextlib
            _mstack = contextlib.ExitStack()
            aT_pool = _mstack.enter_context(
                tc.tile_pool(name="aT", bufs=GROUP // 2 + 2))
            chainx = _mstack.enter_context(
                tc.tile_pool(name="chainx", bufs=int(os.environ.get("DD_CHX", "3"))))
            px4_pool = _mstack.enter_context(
                tc.tile_pool(name="px4", bufs=int(os.environ.get("DD_PX4", "2")), space="PSUM"))
            pchain = _mstack.enter_context(
                tc.tile_pool(name="pchain", bufs=int(os.environ.get("DD_PCH", "2")), space="PSUM"))

            act_prev = [None]
            nopin = bool(int(os.environ.get("DD_NOPIN", "0")))

            def act_chain(bi):
                if act_prev[0] is not None and not nopin:
                    from concourse.tile_rust import add_dep_helper
                    add_dep_helper(bi.ins, act_prev[0].ins, sync=True,
                                   reason="pin ACT order for act-table reuse")
                act_prev[0] = bi
                return bi

            row_stage = {}            # r_loc -> (ssa tile, r3 tile)

            def stage_row(r_loc):
                if r_loc in row_stage:
                    return row_stage[r_loc]
                # rstd-scaled SA: ssa[p, jt, s] = rstd[i, jt*128+p] * SA[jt*128+p, s]
                ssa = ssa_pool.tile([128, NBLK, 256], f16, tag="ssa", name="ssa")
                B = r_loc // RPB  # this row's block: only jt >= B is used
                nc.vector.tensor_mul(
                    ssa[:, B:, :], sa16[:, B:, :],
                    rstd_T[:, B:, r_loc:r_loc + 1].to_broadcast([128, NBLK - B, 256]))
                rb = prow.tile([3, NB + 256], f16, tag="rb", name="rb")
                nc.sync.dma_start(out=rb, in_=row_scratch[:, r_loc, :])
                row_stage[r_loc] = (ssa, rb)
                return row_stage[r_loc]

            def ensure_row(r_loc):
                res = stage_row(r_loc)
                for ahead in (1, 2):       # prefetch upcoming rows
                    if r_loc + ahead < NROWS:
                        stage_row(r_loc + ahead)
                return res

            def score_chunk(c):
                """scores + softmax for tiles 4c..4c+3 -> attn tile (f16)."""
                px4 = px4_pool.tile([128, 4, 256], f32, tag="px4", name="px4")
                metas = []
                for ti in range(4):
                    B, r, jt = TILES[4 * c + ti]
                    r_loc = B * RPB + r
                    ssa, rb = ensure_row(r_loc)
                    metas.append((r_loc, jt, ssa))
                # per psum bank: rank-3 stats then the rstd*SA_j adds
                for h in range(2):
                    for q in range(2):
                        r_loc, jt, ssa = metas[2 * h + q]
                        _, rb = row_stage[r_loc]
                        nc.tensor.matmul(px4[:, 2 * h + q, :],
                                         rb[:, jt * 128:jt * 128 + 128],
                                         rb[:, NB:],
                                         start=(q == 0), stop=False,
                                         skip_group_check=True)
                    m0, m1 = metas[2 * h], metas[2 * h + 1]
                    last = (h == 1)
                    if m0[0] == m1[0] and m1[1] == m0[1] + 1:
                        nc.tensor.matmul(
                            px4[:, 2 * h:2 * h + 2, :].rearrange("p a s -> p (a s)"),
                            ident16,
                            m0[2][:, m0[1]:m0[1] + 2, :].rearrange("p a s -> p (a s)"),
                            start=False, stop=last, skip_group_check=True)
                    else:
                        for q in range(2):
                            r_loc, jt, ssa = metas[2 * h + q]
                            nc.tensor.matmul(px4[:, 2 * h + q, :], ident16,
                                             ssa[:, jt, :],
                                             start=False, stop=last and q == 1,
                                             skip_group_check=True)
                ee = ee_pool.tile([128, 4, 8, 32], f32, tag="ee", name="ee")
                act_chain(nc.scalar.activation(
                    out=ee.rearrange("p a h t -> p (a h t)"),
                    in_=px4.rearrange("p a s -> p (a s)"),
                    func=AF.Exp))
                den = small.tile([128, 4, 8], f32, tag="den", name="den")
                nc.vector.reduce_sum(out=den, in_=ee, axis=AX.X)
                rden = small.tile([128, 4, 8], f32, tag="rden", name="rden")
                nc.vector.reciprocal(out=rden, in_=den)
                attn = attn_pool.tile([128, 4, 8, 32], f16, tag="attn",
                                      name="attn")
                nc.gpsimd.tensor_mul(attn, ee,
                                     rden.to_broadcast([128, 4, 8, 32]))
                return attn

            def chain_cp(aT, cp):
                # aT [128 s, 2 qi, 8 blk=(tile,kt), 128 pair]
                aTr = aT.rearrange("p q (a k) f -> p q a k f", k=2)

                def layer(x_of, w, b_tile, out_tile):
                    for mt in range(2):
                        ps = pchain.tile([128, 2, 512], f32, tag="pch",
                                         name="pch")
                        for kt in range(2):
                            for qi in range(2):
                                nc.tensor.matmul(
                                    ps[:, qi, :],
                                    w[:, kt, mt * 128:(mt + 1) * 128],
                                    x_of(qi, kt),
                                    start=(kt == 0), stop=(kt == 1))
                        act_chain(nc.scalar.activation(
                            out=out_tile[:, mt, :],
                            in_=ps.rearrange("p q n -> p (q n)"), func=AF.Silu,
                            bias=b_tile[:, mt:mt + 1]))

                x2 = chainx.tile([128, 2, 1024], f16, tag="x", name="x2")
                layer(lambda qi, kt: aTr[:, qi, :, kt, :], w_a, b_a, x2)
                x4 = chainx.tile([128, 2, 1024], f16, tag="x", name="x4")
                layer(lambda qi, kt: x2[:, kt, qi * 512:(qi + 1) * 512],
                      w_b, b_b, x4)
                x5 = chainx.tile([128, 2, 1024], f16, tag="x", name="x5")
                layer(lambda qi, kt: x4[:, kt, qi * 512:(qi + 1) * 512],
                      w_d1, b_d1, x5)
                ps6 = pchain.tile([2, 2, 512], f32, tag="pch", name="ps6")
                for kt in range(2):
                    for qi in range(2):
                        nc.tensor.matmul(ps6[:, qi, :], w_d2[:, kt, :],
                                         x5[:, kt, qi * 512:(qi + 1) * 512],
                                         start=(kt == 0), stop=(kt == 1))
                # bias bd2 is added host-side during assembly
                o6 = outp.tile([2, 2, 512], f32, tag="o6", name="o6")
                nc.vector.tensor_copy(out=o6, in_=ps6)
                nc.sync.dma_start(
                    out=out_ext[cp],
                    in_=o6.rearrange("f q n -> f (q n)"))

            stage = int(os.environ.get("DD_STAGE", "9"))
            if stage < 2:
                dummy = outp.tile([2, 2, 512], f32, tag="o6", name="dummy")
                nc.vector.memset(dummy, 0.5)
                for q in range(n_cps):
                    nc.sync.dma_start(out=out_ext[q],
                                      in_=dummy.rearrange("f q n -> f (q n)"))
            else:
                n_super = (n_chunks + GROUP - 1) // GROUP
                pending = []
                for sc in range(n_super):
                    qs = list(range(sc * GROUP, min((sc + 1) * GROUP, n_chunks)))
                    ready = []
                    aT = None
                    for k, c in enumerate(qs):
                        if k % 2 == 0:
                            aT = aT_pool.tile([128, 2, 8, 128], f16, tag="aT",
                                              name="aT")
                        attn = score_chunk(c)
                        nc.sync.dma_start(
                            out=aT[:, k % 2],
                            in_=attn.rearrange("p a h t -> p (a h t)"),
                            transpose=True)
                        if k % 2 == 1:
                            ready.append((aT, c // 2))
                    for aTp, cp in pending:
                        chain_cp(aTp, cp)
                    pending = ready
                for aTp, cp in pending:
                    chain_cp(aTp, cp)
            _mstack.close()
    nc.compile()
    return nc


def _get_nc(n_chunks):
    key = ("nc", n_chunks)
    if key not in _CACHE:
        _CACHE[key] = _build_nc(n_chunks)
    return _CACHE[key]


def kernel(**inputs):
    from concourse.bass_utils import run_bass_kernel_spmd

    n_chunks = int(os.environ.get("DD_CHUNKS", NCHUNKS))
    pc = _precompute(inputs)
    in_maps = [_core_inputs(pc, c) for c in range(NCORES)]
    nc = _get_nc(n_chunks)
    res = run_bass_kernel_spmd(nc, in_maps, core_ids=list(range(NCORES)),
                               trace=bool(int(os.environ.get("DD_TRACE", "0"))))
    _CACHE["last_result"] = res

    R = np.zeros((NB, NB, 2), np.float32)
    for c in range(NCORES):
        o = res.results[c]["out"] + pc["bd2"][None, :, None]   # (NCPS, 2, 1024)
        ot = o.reshape(NCPS, 2, CPT, 128).transpose(0, 2, 1, 3).reshape(-1, 2, 128)
        for t in range(n_chunks * CHUNK):
            B, r, jt = TILES[t]
            i = B * 128 + r * NCORES + c
            R[i, jt * 128:(jt + 1) * 128, 0] = ot[t, 0]
            R[i, jt * 128:(jt + 1) * 128, 1] = ot[t, 1]
    for bi in range(NBLK):
        for bj in range(bi):
            R[bi * 128:(bi + 1) * 128, bj * 128:(bj + 1) * 128] = \
                R[bj * 128:(bj + 1) * 128, bi * 128:(bi + 1) * 128].transpose(1, 0, 2)

    rho = (R[:, :, 0] + 1j * R[:, :, 1]).astype(np.complex64)
    n_spin = int(np.asarray(inputs["n_spin"]))
    return np.broadcast_to(rho[None], (n_spin, NB, NB)).copy()


# revision 7
# speedup vs baseline: 1.3355x; 1.1101x over previous
"""Trainium2 Bass kernel for nn_DensityDecoder (gnn_message_passing).

Math: for every ordered pair (i, j) of NB=640 orbitals,
    pair = orb_i + orb_j
    qn   = LayerNorm(pair) ; q = qn @ Wq + bq
    attn = softmax(q . k / sqrt(Dh)) over a tiny T=32 latent KV
    out  = MLP(attn @ V @ Wo)  ->  2 values -> rho[i, j] = out0 + 1j*out1

LN statistics decompose exactly over pair = orb_i + orb_j, so the whole
pre-softmax pipeline collapses to per-orbital precomputes projected into
(head, token) score space:
    scores_ij = rstd_ij * (SA_i + SA_j - mu_ij*Sw) + Sb      (pre-scaled 1/sqrt(Dh))

Per 4-tile chunk the scores land in one [128, 4, 256] PSUM tile (rank-3
stats matmuls + rstd-scaled-SA identity adds), one Exp covers the chunk,
and the attn -> attn^T reshuffle for the feature chain runs on the DMA
XBAR transpose (16x128 tiles) instead of PE transpose matmuls, writing
fp16 straight into SBUF in [s, (tile,kt), pair] block layout.  The MLP
chain runs fp16 end to end (weights + activations; PSUM accumulation is
fp32).

rho is symmetric; only j-blocks >= i-block are computed (240 of 400 tiles),
the lower triangle is mirrored host-side.

Sharding: rows i striped across 8 cores (i % 8 == core): identical SPMD
instruction stream, 80 rows -> 240 tiles of 128 pairs -> 30 chain-pairs.
"""

import os
import numpy as np

EPS = 1e-5
H = 8
D = 256
T = 32
Dh = D // H
NB = 640
NCORES = 8
NBLK = NB // 128          # 5 column blocks
RPB = 128 // NCORES       # 16 rows per block per core
NROWS = NBLK * RPB        # 80 rows per core
TILES = [(B, r, jt) for B in range(NBLK) for r in range(RPB) for jt in range(B, NBLK)]
NTILES = len(TILES)       # 240
CHUNK = 4                 # tiles per score chunk (one [128, 4, 256] psum)
CPT = 2 * CHUNK           # tiles per chain-pair (2 chunks)
GROUP = int(os.environ.get("DD_GROUP", "8"))  # chunks per superchunk
NCHUNKS = NTILES // CHUNK  # 60
NCPS = NTILES // CPT       # 30

_CACHE = {}


def _silu(x):
    return x / (1.0 + np.exp(-x))


def _ln(x, g, b):
    mu = x.mean(-1, keepdims=True)
    var = x.var(-1, keepdims=True)
    return (x - mu) / np.sqrt(var + EPS) * g + b


def _precompute(inputs):
    """Pair-independent precompute (all O(NB*D) or smaller)."""
    f = {}
    for k, v in inputs.items():
        v = np.asarray(v)
        f[k] = v.astype(np.float64) if v.dtype in (np.float32, np.float64) else v
    Z = np.asarray(inputs["Z"]).astype(np.int64)
    l = np.asarray(inputs["l"]).astype(np.int64)
    m = np.asarray(inputs["m"]).astype(np.int64)
    m_idx = np.clip(m + 3, 0, 4)
    emb = np.concatenate([f["elem_tab"][Z], f["l_tab"][l], f["m_tab"][m_idx]], -1)
    orb = _silu(emb @ f["Wp0"] + f["bp0"]) @ f["Wp1"] + f["bp1"]          # (NB, D)

    kv = _ln(f["latent"], f["ln_gkv"], f["ln_bkv"])
    k = (kv @ f["Wk"] + f["bk"]).reshape(T, H, Dh)
    v = (kv @ f["Wv"] + f["bv"]).reshape(T, H, Dh)

    g, b = f["ln_gq"], f["ln_bq"]
    mu = orb.mean(-1)
    msq = (orb * orb).mean(-1)

    A = (orb * g) @ f["Wq"]
    wbar = g @ f["Wq"]
    bq_eff = b @ f["Wq"] + f["bqa"]

    kT = k.transpose(1, 2, 0)                                            # (H, Dh, T)
    scale = 1.0 / np.sqrt(np.float64(Dh))

    def to_scores(x):
        xh = x.reshape(x.shape[:-1] + (H, Dh))
        return (np.einsum('...hd,hdt->...ht', xh, kT).reshape(x.shape[:-1] + (H * T,))
                * scale)

    SA = to_scores(A)                                                    # (NB, 256)
    Sw = to_scores(wbar)                                                 # (256,)
    Sb = to_scores(bq_eff)                                               # (256,)
    Wvo = np.einsum('thd,hde->hte', v, f["Wo"].reshape(H, Dh, D)).reshape(H * T, D)
    # fuse consecutive linear layers (no nonlinearity between them)
    Wa = Wvo @ f["Wt0"]
    ba = f["bo"] @ f["Wt0"] + f["bt0"]
    Wb = f["Wt1"] @ f["Wd0"]
    bb = f["bt1"] @ f["Wd0"] + f["bd0"]

    fl = lambda x: np.ascontiguousarray(x, np.float32)
    return {
        "SA": fl(SA), "Sw": fl(Sw), "Sb": fl(Sb), "mu": fl(mu), "msq": fl(msq),
        "orbT_s": fl(orb.T * np.sqrt(2.0 / D)),                          # (D, NB)
        "Wa": fl(Wa), "ba": fl(ba), "Wb": fl(Wb), "bb": fl(bb),
        "Wd1": fl(f["Wd1"]), "bd1": fl(f["bd1"]),
        "Wd2": fl(f["Wd2"]), "bd2": fl(f["bd2"]),
    }


def core_rows(c):
    return [B * 128 + r * NCORES + c for B in range(NBLK) for r in range(RPB)]


def _core_inputs(pc, c):
    rows = core_rows(c)
    f16 = np.float16
    # per local row r the rank-3 rhs rows [SA_i; -Sw; Sb]
    r3rows = np.zeros((NROWS, 3, 256), f16)
    for r, i in enumerate(rows):
        r3rows[r, 0] = pc["SA"][i]
        r3rows[r, 1] = -pc["Sw"]
        r3rows[r, 2] = pc["Sb"]
    ones80 = np.ones(NROWS, np.float32)
    return {
        "sa_in": pc["SA"].astype(f16),
        "r3rows_in": r3rows,
        "orbT_in": pc["orbT_s"],
        "orbTc_in": np.ascontiguousarray(pc["orbT_s"][:, rows]),
        "lhs_mu": np.ascontiguousarray(np.stack([ones80, pc["mu"][rows]])),
        "lhs_msq": np.ascontiguousarray(np.stack([ones80, pc["msq"][rows]])),
        "rhs_mu": np.ascontiguousarray(np.stack([pc["mu"], np.ones(NB, np.float32)])),
        "rhs_msq": np.ascontiguousarray(np.stack([pc["msq"], np.ones(NB, np.float32)])),
        "ident_in": np.eye(128, dtype=np.float32),
        "ident16_in": np.eye(128, dtype=f16),
        "wa": pc["Wa"].astype(f16), "wb": pc["Wb"].astype(f16),
        "wd1": pc["Wd1"].astype(f16), "wd2": pc["Wd2"].astype(f16),
        "ba_in": pc["ba"], "bb_in": pc["bb"], "bd1_in": pc["bd1"],
    }


def _build_nc(n_chunks):
    import concourse.bass as bass
    import concourse.bacc as bacc
    import concourse.tile as tile
    from concourse import mybir
    dt = mybir.dt
    f32 = dt.float32
    f32r = dt.float32r
    f16 = dt.float16
    AF = mybir.ActivationFunctionType
    AX = mybir.AxisListType

    assert n_chunks % 2 == 0
    n_cps = n_chunks // 2

    nc = bacc.Bacc(None, target_bir_lowering=False)

    ein = lambda name, shape, d=f32: nc.dram_tensor(name, shape, d,
                                                     kind="ExternalInput")
    sa_in = ein("sa_in", [NB, 256], f16)
    r3rows_in = ein("r3rows_in", [NROWS, 3, 256], f16)
    orbT_in = ein("orbT_in", [D, NB], f32r)
    orbTc_in = ein("orbTc_in", [D, NROWS], f32r)
    lhs_mu = ein("lhs_mu", [2, NROWS], f32r)
    lhs_msq = ein("lhs_msq", [2, NROWS], f32r)
    rhs_mu = ein("rhs_mu", [2, NB], f32r)
    rhs_msq = ein("rhs_msq", [2, NB], f32r)
    ident_in = ein("ident_in", [128, 128], f32r)
    ident16_in = ein("ident16_in", [128, 128], f16)
    wa = ein("wa", [256, 256], f16)
    wb = ein("wb", [256, 256], f16)
    wd1 = ein("wd1", [256, 256], f16)
    wd2 = ein("wd2", [256, 2], f16)
    ba_in = ein("ba_in", [256])
    bb_in = ein("bb_in", [256])
    bd1_in = ein("bd1_in", [256])

    out_ext = nc.dram_tensor("out", [NCPS, 2, 1024], f32, kind="ExternalOutput")
    # combined per-row operands: [rstd | SA_i; rstd*mu | -Sw; ones | Sb]
    row_scratch = nc.dram_tensor("row_scratch", [3, NROWS, NB + 256], f16)

    with tile.TileContext(nc) as tc, \
            nc.allow_low_precision(reason="fp16 pipeline by design"):
        with (
            tc.tile_pool(name="const", bufs=1) as const,
            tc.tile_pool(name="ssa", bufs=int(os.environ.get("DD_SSA", "12"))) as ssa_pool,
            tc.tile_pool(name="prow", bufs=int(os.environ.get("DD_PROW", "12"))) as prow,
            tc.tile_pool(name="ee", bufs=int(os.environ.get("DD_EE", "6"))) as ee_pool,
            tc.tile_pool(name="attnp", bufs=int(os.environ.get("DD_ATTN", "8"))) as attn_pool,
            tc.tile_pool(name="small", bufs=4) as small,
            tc.tile_pool(name="outp", bufs=2) as outp,
        ):
            # ---- constants into SBUF ----
            sa16 = const.tile([128, NBLK, 256], f16)
            nc.sync.dma_start(out=sa16, in_=sa_in.rearrange("(jt p) c -> p jt c", p=128))
            orbT = const.tile([128, 2, NB], f32r)
            nc.sync.dma_start(out=orbT, in_=orbT_in.rearrange("(k p) n -> p k n", p=128))
            orbTc = const.tile([128, 2, NROWS], f32r)
            nc.sync.dma_start(out=orbTc, in_=orbTc_in.rearrange("(k p) m -> p k m", p=128))
            lmu = const.tile([2, NROWS], f32r)
            nc.sync.dma_start(out=lmu, in_=lhs_mu[:])
            lmsq = const.tile([2, NROWS], f32r)
            nc.sync.dma_start(out=lmsq, in_=lhs_msq[:])
            rmu = const.tile([2, NB], f32r)
            nc.sync.dma_start(out=rmu, in_=rhs_mu[:])
            rmsq = const.tile([2, NB], f32r)
            nc.sync.dma_start(out=rmsq, in_=rhs_msq[:])

            w_a = const.tile([128, 2, 256], f16)
            nc.sync.dma_start(out=w_a, in_=wa.rearrange("(k p) n -> p k n", p=128))
            w_b = const.tile([128, 2, 256], f16)
            nc.sync.dma_start(out=w_b, in_=wb.rearrange("(k p) n -> p k n", p=128))
            w_d1 = const.tile([128, 2, 256], f16)
            nc.sync.dma_start(out=w_d1, in_=wd1.rearrange("(k p) n -> p k n", p=128))
            w_d2 = const.tile([128, 2, 2], f16)
            nc.sync.dma_start(out=w_d2, in_=wd2.rearrange("(k p) n -> p k n", p=128))

            b_a = const.tile([128, 2], f32)
            nc.sync.dma_start(out=b_a, in_=ba_in.rearrange("(m p) -> p m", p=128))
            b_b = const.tile([128, 2], f32)
            nc.sync.dma_start(out=b_b, in_=bb_in.rearrange("(m p) -> p m", p=128))
            b_d1 = const.tile([128, 2], f32)
            nc.sync.dma_start(out=b_d1, in_=bd1_in.rearrange("(m p) -> p m", p=128))

            ident = const.tile([128, 128], f32r)
            nc.sync.dma_start(out=ident, in_=ident_in[:])
            ident16 = const.tile([128, 128], f16)
            nc.sync.dma_start(out=ident16, in_=ident16_in[:])
            eps_t = const.tile([NROWS, 1], f32)
            nc.vector.memset(eps_t, EPS)

            rstd_T = const.tile([128, NBLK, NROWS], f32)

            # ---- prologue: per-pair LN stats for this core's 80 rows ----
            with (
                tc.tile_pool(name="pro_ps", bufs=2, space="PSUM") as pro_ps,
                tc.tile_pool(name="pro_sb", bufs=1) as pro_sb,
            ):
                mu_p_sb = pro_sb.tile([NROWS, NB], f32r, tag="mu_p")
                rstd_sb = pro_sb.tile([NROWS, NB], f32r, tag="rstd")
                invr_sb = pro_sb.tile([NROWS, NB], f32r, tag="invr")
                for nch in range(2):
                    seg = slice(nch * 320, (nch + 1) * 320)
                    psA = pro_ps.tile([NROWS, 320], f32, tag="psA")
                    nc.tensor.matmul(psA, lmu, rmu[:, seg], start=True, stop=True)
                    nc.vector.tensor_copy(out=mu_p_sb[:, seg], in_=psA)
                    psB = pro_ps.tile([NROWS, 320], f32, tag="psB")
                    nc.tensor.matmul(psB, lmsq, rmsq[:, seg], start=True, stop=False)
                    nc.tensor.matmul(psB, orbTc[:, 0, :], orbT[:, 0, seg],
                                     start=False, stop=False)
                    nc.tensor.matmul(psB, orbTc[:, 1, :], orbT[:, 1, seg],
                                     start=False, stop=True)
                    mu2 = pro_sb.tile([NROWS, 320], f32, tag="mu2")
                    nc.vector.tensor_mul(mu2, mu_p_sb[:, seg], mu_p_sb[:, seg])
                    nc.vector.tensor_sub(invr_sb[:, seg], psB, mu2)
                # invr = sqrt(var + eps); rstd = 1/invr
                nc.scalar.activation(out=invr_sb, in_=invr_sb, func=AF.Sqrt,
                                     bias=eps_t[:, 0:1])
                nc.vector.reciprocal(out=rstd_sb, in_=invr_sb)
                # rstd*mu_p, and fp16 casts of both rows
                rstdmu = pro_sb.tile([NROWS, NB], f32, tag="rstdmu")
                nc.vector.tensor_mul(rstdmu, rstd_sb, mu_p_sb)
                rstd16 = pro_sb.tile([NROWS, NB], f16, tag="rstd16")
                nc.vector.tensor_copy(out=rstd16, in_=rstd_sb)
                rstdmu16 = pro_sb.tile([NROWS, NB], f16, tag="rstdmu16")
                nc.vector.tensor_copy(out=rstdmu16, in_=rstdmu)
                # assemble the combined per-row operand planes in DRAM
                nc.sync.dma_start(out=row_scratch[0, :, 0:NB], in_=rstd16)
                nc.sync.dma_start(out=row_scratch[1, :, 0:NB], in_=rstdmu16)
                ones16 = pro_sb.tile([NROWS, NB], f16, tag="ones16")
                nc.vector.memset(ones16, 1.0)
                nc.sync.dma_start(out=row_scratch[2, :, 0:NB], in_=ones16)
                nc.sync.dma_start(out=row_scratch[:, :, NB:],
                                  in_=r3rows_in.rearrange("q k n -> k q n"))
                # transposed rstd for the per-row scaled-SA products
                for jt in range(NBLK):
                    pT = pro_ps.tile([128, NROWS], f32r, tag="pT")
                    nc.tensor.transpose(
                        pT, rstd_sb[:, jt * 128:(jt + 1) * 128],
                        ident[0:NROWS, 0:NROWS])
                    nc.vector.tensor_copy(out=rstd_T[:, jt, :], in_=pT)

            # ---- main loop ----
            import contextlib
            _mstack = contextlib.ExitStack()
            aT_pool = _mstack.enter_context(
                tc.tile_pool(name="aT", bufs=GROUP // 2 + 2))
            chainx = _mstack.enter_context(
                tc.tile_pool(name="chainx", bufs=int(os.environ.get("DD_CHX", "3"))))
            px4_pool = _mstack.enter_context(
                tc.tile_pool(name="px4", bufs=int(os.environ.get("DD_PX4", "2")), space="PSUM"))
            pchain = _mstack.enter_context(
                tc.tile_pool(name="pchain", bufs=int(os.environ.get("DD_PCH", "2")), space="PSUM"))

            act_prev = [None]
            nopin = bool(int(os.environ.get("DD_NOPIN", "0")))

            def act_chain(bi):
                if act_prev[0] is not None and not nopin:
                    from concourse.tile_rust import add_dep_helper
                    add_dep_helper(bi.ins, act_prev[0].ins, sync=True,
                                   reason="pin ACT order for act-table reuse")
                act_prev[0] = bi
                return bi

            row_stage = {}            # r_loc -> (ssa tile, r3 tile)

            def stage_row(r_loc):
                if r_loc in row_stage:
                    return row_stage[r_loc]
                # rstd-scaled SA: ssa[p, jt, s] = rstd[i, jt*128+p] * SA[jt*128+p, s]
                # one tensor_scalar per block keeps the DVE 2x perf modes
                ssa = ssa_pool.tile([128, NBLK, 256], f16, tag="ssa", name="ssa")
                B = r_loc // RPB  # this row's block: only jt >= B is used
                for jt in range(B, NBLK):
                    nc.vector.tensor_scalar_mul(
                        ssa[:, jt, :], sa16[:, jt, :],
                        rstd_T[:, jt, r_loc:r_loc + 1])
                rb = prow.tile([3, NB + 256], f16, tag="rb", name="rb")
                nc.sync.dma_start(out=rb, in_=row_scratch[:, r_loc, :])
                row_stage[r_loc] = (ssa, rb)
                return row_stage[r_loc]

            def ensure_row(r_loc):
                res = stage_row(r_loc)
                for ahead in (1, 2):       # prefetch upcoming rows
                    if r_loc + ahead < NROWS:
                        stage_row(r_loc + ahead)
                return res

            def score_chunk(c):
                """scores + softmax for tiles 4c..4c+3 -> attn tile (f16)."""
                px4 = px4_pool.tile([128, 4, 256], f32, tag="px4", name="px4")
                metas = []
                for ti in range(4):
                    B, r, jt = TILES[4 * c + ti]
                    r_loc = B * RPB + r
                    ssa, rb = ensure_row(r_loc)
                    metas.append((r_loc, jt, ssa))
                # per psum bank: rank-3 stats then the rstd*SA_j adds
                for h in range(2):
                    for q in range(2):
                        r_loc, jt, ssa = metas[2 * h + q]
                        _, rb = row_stage[r_loc]
                        nc.tensor.matmul(px4[:, 2 * h + q, :],
                                         rb[:, jt * 128:jt * 128 + 128],
                                         rb[:, NB:],
                                         start=(q == 0), stop=False,
                                         skip_group_check=True)
                    m0, m1 = metas[2 * h], metas[2 * h + 1]
                    last = (h == 1)
                    if m0[0] == m1[0] and m1[1] == m0[1] + 1:
                        nc.tensor.matmul(
                            px4[:, 2 * h:2 * h + 2, :].rearrange("p a s -> p (a s)"),
                            ident16,
                            m0[2][:, m0[1]:m0[1] + 2, :].rearrange("p a s -> p (a s)"),
                            start=False, stop=last, skip_group_check=True)
                    else:
                        for q in range(2):
                            r_loc, jt, ssa = metas[2 * h + q]
                            nc.tensor.matmul(px4[:, 2 * h + q, :], ident16,
                                             ssa[:, jt, :],
                                             start=False, stop=last and q == 1,
                                             skip_group_check=True)
                ee = ee_pool.tile([128, 4, 8, 32], f16, tag="ee", name="ee")
                act_chain(nc.scalar.activation(
                    out=ee.rearrange("p a h t -> p (a h t)"),
                    in_=px4.rearrange("p a s -> p (a s)"),
                    func=AF.Exp))
                den = small.tile([128, 4, 8], f16, tag="den", name="den")
                nc.vector.reduce_sum(out=den, in_=ee, axis=AX.X)
                rden = small.tile([128, 4, 8], f16, tag="rden", name="rden")
                nc.vector.reciprocal(out=rden, in_=den)
                attn = attn_pool.tile([128, 4, 8, 32], f16, tag="attn",
                                      name="attn")
                nc.gpsimd.tensor_mul(attn, ee,
                                     rden.to_broadcast([128, 4, 8, 32]))
                return attn

            def chain_cp(aT, cp):
                # aT [128 s, 2 qi, 8 blk=(tile,kt), 128 pair]
                aTr = aT.rearrange("p q (a k) f -> p q a k f", k=2)

                def layer(x_of, w, b_tile, out_tile):
                    for mt in range(2):
                        ps = pchain.tile([128, 2, 512], f32, tag="pch",
                                         name="pch")
                        for kt in range(2):
                            for qi in range(2):
                                nc.tensor.matmul(
                                    ps[:, qi, :],
                                    w[:, kt, mt * 128:(mt + 1) * 128],
                                    x_of(qi, kt),
                                    start=(kt == 0), stop=(kt == 1))
                        act_chain(nc.scalar.activation(
                            out=out_tile[:, mt, :],
                            in_=ps.rearrange("p q n -> p (q n)"), func=AF.Silu,
                            bias=b_tile[:, mt:mt + 1]))

                x2 = chainx.tile([128, 2, 1024], f16, tag="x", name="x2")
                layer(lambda qi, kt: aTr[:, qi, :, kt, :], w_a, b_a, x2)
                x4 = chainx.tile([128, 2, 1024], f16, tag="x", name="x4")
                layer(lambda qi, kt: x2[:, kt, qi * 512:(qi + 1) * 512],
                      w_b, b_b, x4)
                x5 = chainx.tile([128, 2, 1024], f16, tag="x", name="x5")
                layer(lambda qi, kt: x4[:, kt, qi * 512:(qi + 1) * 512],
                      w_d1, b_d1, x5)
                ps6 = pchain.tile([2, 2, 512], f32, tag="pch", name="ps6")
                for kt in range(2):
                    for qi in range(2):
                        nc.tensor.matmul(ps6[:, qi, :], w_d2[:, kt, :],
                                         x5[:, kt, qi * 512:(qi + 1) * 512],
                                         start=(kt == 0), stop=(kt == 1))
                # bias bd2 is added host-side during assembly
                o6 = outp.tile([2, 2, 512], f32, tag="o6", name="o6")
                nc.vector.tensor_copy(out=o6, in_=ps6)
                nc.sync.dma_start(
                    out=out_ext[cp],
                    in_=o6.rearrange("f q n -> f (q n)"))

            stage = int(os.environ.get("DD_STAGE", "9"))
            if stage < 2:
                dummy = outp.tile([2, 2, 512], f32, tag="o6", name="dummy")
                nc.vector.memset(dummy, 0.5)
                for q in range(n_cps):
                    nc.sync.dma_start(out=out_ext[q],
                                      in_=dummy.rearrange("f q n -> f (q n)"))
            else:
                n_super = (n_chunks + GROUP - 1) // GROUP
                pstage = int(os.environ.get("DD_PSTAGE", "8"))

                def rows_of_super(sc):
                    rows = []
                    for c in range(sc * GROUP, min((sc + 1) * GROUP, n_chunks)):
                        for ti in range(4):
                            B, r, jt = TILES[4 * c + ti]
                            r_loc = B * RPB + r
                            if r_loc not in rows:
                                rows.append(r_loc)
                    return rows

                pending = []
                for sc in range(n_super):
                    qs = list(range(sc * GROUP, min((sc + 1) * GROUP, n_chunks)))
                    ready = []
                    aT = None
                    for k, c in enumerate(qs):
                        if k % 2 == 0:
                            aT = aT_pool.tile([128, 2, 8, 128], f16, tag="aT",
                                              name="aT")
                        attn = score_chunk(c)
                        nc.sync.dma_start(
                            out=aT[:, k % 2],
                            in_=attn.rearrange("p a h t -> p (a h t)"),
                            transpose=True)
                        if k % 2 == 1:
                            ready.append((aT, c // 2))
                    # chains of the previous super, interleaved with staging
                    # of the next super's rows: the DVE staging muls land in
                    # the chain window where the vector engine is idle
                    nxt = [r for r in rows_of_super(sc + 1)
                           if r not in row_stage][:pstage] if sc + 1 < n_super else []
                    per = -(-len(nxt) // max(1, len(pending))) if pending else 0
                    for ci, (aTp, cp) in enumerate(pending):
                        chain_cp(aTp, cp)
                        for r in nxt[ci * per:(ci + 1) * per]:
                            stage_row(r)
                    for r in (nxt[len(pending) * per:] if pending else nxt):
                        stage_row(r)
                    pending = ready
                for aTp, cp in pending:
                    chain_cp(aTp, cp)
            _mstack.close()
    nc.compile()
    return nc


def _get_nc(n_chunks):
    key = ("nc", n_chunks)
    if key not in _CACHE:
        _CACHE[key] = _build_nc(n_chunks)
    return _CACHE[key]


def kernel(**inputs):
    from concourse.bass_utils import run_bass_kernel_spmd

    n_chunks = int(os.environ.get("DD_CHUNKS", NCHUNKS))
    pc = _precompute(inputs)
    in_maps = [_core_inputs(pc, c) for c in range(NCORES)]
    nc = _get_nc(n_chunks)
    res = run_bass_kernel_spmd(nc, in_maps, core_ids=list(range(NCORES)),
                               trace=bool(int(os.environ.get("DD_TRACE", "0"))))
    _CACHE["last_result"] = res

    R = np.zeros((NB, NB, 2), np.float32)
    for c in range(NCORES):
        o = res.results[c]["out"] + pc["bd2"][None, :, None]   # (NCPS, 2, 1024)
        ot = o.reshape(NCPS, 2, CPT, 128).transpose(0, 2, 1, 3).reshape(-1, 2, 128)
        for t in range(n_chunks * CHUNK):
            B, r, jt = TILES[t]
            i = B * 128 + r * NCORES + c
            R[i, jt * 128:(jt + 1) * 128, 0] = ot[t, 0]
            R[i, jt * 128:(jt + 1) * 128, 1] = ot[t, 1]
    for bi in range(NBLK):
        for bj in range(bi):
            R[bi * 128:(bi + 1) * 128, bj * 128:(bj + 1) * 128] = \
                R[bj * 128:(bj + 1) * 128, bi * 128:(bi + 1) * 128].transpose(1, 0, 2)

    rho = (R[:, :, 0] + 1j * R[:, :, 1]).astype(np.complex64)
    n_spin = int(np.asarray(inputs["n_spin"]))
    return np.broadcast_to(rho[None], (n_spin, NB, NB)).copy()


# revision 10
# speedup vs baseline: 1.6026x; 1.2000x over previous
"""Trainium2 Bass kernel for nn_DensityDecoder (gnn_message_passing).

Math: for every ordered pair (i, j) of NB=640 orbitals,
    pair = orb_i + orb_j
    qn   = LayerNorm(pair) ; q = qn @ Wq + bq
    attn = softmax(q . k / sqrt(Dh)) over a tiny T=32 latent KV
    out  = MLP(attn @ V @ Wo)  ->  2 values -> rho[i, j] = out0 + 1j*out1

LN statistics decompose exactly over pair = orb_i + orb_j, so the whole
pre-softmax pipeline collapses to per-orbital precomputes projected into
(head, token) score space:
    scores_ij = rstd_ij * (SA_i + SA_j - mu_ij*Sw) + Sb      (pre-scaled 1/sqrt(Dh))

Per 4-tile chunk the scores land in one [128, 4, 256] PSUM tile (rank-3
stats matmuls + rstd-scaled-SA identity adds), one Exp covers the chunk,
and the attn -> attn^T reshuffle for the feature chain runs on the DMA
XBAR transpose (16x128 tiles) instead of PE transpose matmuls, writing
fp16 straight into SBUF in [s, (tile,kt), pair] block layout.  The MLP
chain runs fp16 end to end (weights + activations; PSUM accumulation is
fp32).

rho is symmetric; only j-blocks >= i-block are computed (240 of 400 tiles),
the lower triangle is mirrored host-side.

Sharding: rows i striped across 8 cores (i % 8 == core): identical SPMD
instruction stream, 80 rows -> 240 tiles of 128 pairs -> 30 chain-pairs.
"""

import os
import numpy as np

EPS = 1e-5
H = 8
D = 256
T = 32
Dh = D // H
NB = 640
NCORES = 8
NBLK = NB // 128          # 5 column blocks
RPB = 128 // NCORES       # 16 rows per block per core
NROWS = NBLK * RPB        # 80 rows per core
TILES = [(B, r, jt) for B in range(NBLK) for r in range(RPB) for jt in range(B, NBLK)]
NTILES = len(TILES)       # 240
CHUNK = 4                 # tiles per score chunk (one [128, 4, 256] psum)
CPT = 2 * CHUNK           # tiles per chain-pair (2 chunks)
GROUP = int(os.environ.get("DD_GROUP", "8"))  # chunks per superchunk
NCHUNKS = NTILES // CHUNK  # 60
NCPS = NTILES // CPT       # 30

_CACHE = {}


def _silu(x):
    return x / (1.0 + np.exp(-x))


def _ln(x, g, b):
    mu = x.mean(-1, keepdims=True)
    var = x.var(-1, keepdims=True)
    return (x - mu) / np.sqrt(var + EPS) * g + b


def _precompute(inputs):
    """Pair-independent precompute (all O(NB*D) or smaller)."""
    f = {}
    for k, v in inputs.items():
        v = np.asarray(v)
        f[k] = v.astype(np.float64) if v.dtype in (np.float32, np.float64) else v
    Z = np.asarray(inputs["Z"]).astype(np.int64)
    l = np.asarray(inputs["l"]).astype(np.int64)
    m = np.asarray(inputs["m"]).astype(np.int64)
    m_idx = np.clip(m + 3, 0, 4)
    emb = np.concatenate([f["elem_tab"][Z], f["l_tab"][l], f["m_tab"][m_idx]], -1)
    orb = _silu(emb @ f["Wp0"] + f["bp0"]) @ f["Wp1"] + f["bp1"]          # (NB, D)

    kv = _ln(f["latent"], f["ln_gkv"], f["ln_bkv"])
    k = (kv @ f["Wk"] + f["bk"]).reshape(T, H, Dh)
    v = (kv @ f["Wv"] + f["bv"]).reshape(T, H, Dh)

    g, b = f["ln_gq"], f["ln_bq"]
    mu = orb.mean(-1)
    msq = (orb * orb).mean(-1)

    A = (orb * g) @ f["Wq"]
    wbar = g @ f["Wq"]
    bq_eff = b @ f["Wq"] + f["bqa"]

    kT = k.transpose(1, 2, 0)                                            # (H, Dh, T)
    scale = 1.0 / np.sqrt(np.float64(Dh))

    def to_scores(x):
        xh = x.reshape(x.shape[:-1] + (H, Dh))
        return (np.einsum('...hd,hdt->...ht', xh, kT).reshape(x.shape[:-1] + (H * T,))
                * scale)

    SA = to_scores(A)                                                    # (NB, 256)
    Sw = to_scores(wbar)                                                 # (256,)
    Sb = to_scores(bq_eff)                                               # (256,)
    Wvo = np.einsum('thd,hde->hte', v, f["Wo"].reshape(H, Dh, D)).reshape(H * T, D)
    # fuse consecutive linear layers (no nonlinearity between them)
    Wa = Wvo @ f["Wt0"]
    ba = f["bo"] @ f["Wt0"] + f["bt0"]
    Wb = f["Wt1"] @ f["Wd0"]
    bb = f["bt1"] @ f["Wd0"] + f["bd0"]

    fl = lambda x: np.ascontiguousarray(x, np.float32)
    return {
        "SA": fl(SA), "Sw": fl(Sw), "Sb": fl(Sb), "mu": fl(mu), "msq": fl(msq),
        "orbT_s": fl(orb.T * np.sqrt(2.0 / D)),                          # (D, NB)
        "Wa": fl(Wa), "ba": fl(ba), "Wb": fl(Wb), "bb": fl(bb),
        "Wd1": fl(f["Wd1"]), "bd1": fl(f["bd1"]),
        "Wd2": fl(f["Wd2"]), "bd2": fl(f["bd2"]),
    }


def core_rows(c):
    return [B * 128 + r * NCORES + c for B in range(NBLK) for r in range(RPB)]


def _core_inputs(pc, c):
    rows = core_rows(c)
    f16 = np.float16
    # per local row r the rank-3 rhs rows [SA_i; -Sw; Sb]
    r3rows = np.zeros((NROWS, 3, 256), f16)
    for r, i in enumerate(rows):
        r3rows[r, 0] = pc["SA"][i]
        r3rows[r, 1] = -pc["Sw"]
        r3rows[r, 2] = pc["Sb"]
    ones80 = np.ones(NROWS, np.float32)
    return {
        "sa_in": pc["SA"].astype(f16),
        "r3rows_in": r3rows,
        "orbT_in": pc["orbT_s"],
        "orbTc_in": np.ascontiguousarray(pc["orbT_s"][:, rows]),
        "lhs_mu": np.ascontiguousarray(np.stack([ones80, pc["mu"][rows]])),
        "lhs_msq": np.ascontiguousarray(np.stack([ones80, pc["msq"][rows]])),
        "rhs_mu": np.ascontiguousarray(np.stack([pc["mu"], np.ones(NB, np.float32)])),
        "rhs_msq": np.ascontiguousarray(np.stack([pc["msq"], np.ones(NB, np.float32)])),
        "ident_in": np.eye(128, dtype=np.float32),
        "ident16_in": np.eye(128, dtype=f16),
        "wa": pc["Wa"].astype(f16), "wb": pc["Wb"].astype(f16),
        "wd1": pc["Wd1"].astype(f16), "wd2": pc["Wd2"].astype(f16),
        "ba_in": pc["ba"], "bb_in": pc["bb"], "bd1_in": pc["bd1"],
    }


def _build_nc(n_chunks):
    import concourse.bass as bass
    import concourse.bacc as bacc
    import concourse.tile as tile
    from concourse import mybir
    dt = mybir.dt
    f32 = dt.float32
    f32r = dt.float32r
    f16 = dt.float16
    AF = mybir.ActivationFunctionType
    AX = mybir.AxisListType

    assert n_chunks % 2 == 0
    n_cps = n_chunks // 2

    nc = bacc.Bacc(None, target_bir_lowering=False)

    ein = lambda name, shape, d=f32: nc.dram_tensor(name, shape, d,
                                                     kind="ExternalInput")
    sa_in = ein("sa_in", [NB, 256], f16)
    r3rows_in = ein("r3rows_in", [NROWS, 3, 256], f16)
    orbT_in = ein("orbT_in", [D, NB], f32r)
    orbTc_in = ein("orbTc_in", [D, NROWS], f32r)
    lhs_mu = ein("lhs_mu", [2, NROWS], f32r)
    lhs_msq = ein("lhs_msq", [2, NROWS], f32r)
    rhs_mu = ein("rhs_mu", [2, NB], f32r)
    rhs_msq = ein("rhs_msq", [2, NB], f32r)
    ident_in = ein("ident_in", [128, 128], f32r)
    ident16_in = ein("ident16_in", [128, 128], f16)
    wa = ein("wa", [256, 256], f16)
    wb = ein("wb", [256, 256], f16)
    wd1 = ein("wd1", [256, 256], f16)
    wd2 = ein("wd2", [256, 2], f16)
    ba_in = ein("ba_in", [256])
    bb_in = ein("bb_in", [256])
    bd1_in = ein("bd1_in", [256])

    out_ext = nc.dram_tensor("out", [NCPS, 2, 1024], f32, kind="ExternalOutput")
    # combined per-row operands: [rstd | SA_i; rstd*mu | -Sw; ones | Sb]
    row_scratch = nc.dram_tensor("row_scratch", [3, NROWS, NB + 256], f16)

    with tile.TileContext(nc) as tc, \
            nc.allow_low_precision(reason="fp16 pipeline by design"):
        with (
            tc.tile_pool(name="const", bufs=1) as const,
            tc.tile_pool(name="ssa", bufs=int(os.environ.get("DD_SSA", "16"))) as ssa_pool,
            tc.tile_pool(name="prow", bufs=int(os.environ.get("DD_PROW", "16"))) as prow,
            tc.tile_pool(name="ee", bufs=int(os.environ.get("DD_EE", "8"))) as ee_pool,
            tc.tile_pool(name="attnp", bufs=int(os.environ.get("DD_ATTN", "10"))) as attn_pool,
            tc.tile_pool(name="small", bufs=4) as small,
            tc.tile_pool(name="outp", bufs=2) as outp,
        ):
            # ---- constants into SBUF ----
            sa16 = const.tile([128, NBLK, 256], f16)
            nc.sync.dma_start(out=sa16, in_=sa_in.rearrange("(jt p) c -> p jt c", p=128))
            orbT = const.tile([128, 2, NB], f32r)
            nc.sync.dma_start(out=orbT, in_=orbT_in.rearrange("(k p) n -> p k n", p=128))
            orbTc = const.tile([128, 2, NROWS], f32r)
            nc.sync.dma_start(out=orbTc, in_=orbTc_in.rearrange("(k p) m -> p k m", p=128))
            lmu = const.tile([2, NROWS], f32r)
            nc.sync.dma_start(out=lmu, in_=lhs_mu[:])
            lmsq = const.tile([2, NROWS], f32r)
            nc.sync.dma_start(out=lmsq, in_=lhs_msq[:])
            rmu = const.tile([2, NB], f32r)
            nc.sync.dma_start(out=rmu, in_=rhs_mu[:])
            rmsq = const.tile([2, NB], f32r)
            nc.sync.dma_start(out=rmsq, in_=rhs_msq[:])

            w_a = const.tile([128, 2, 256], f16)
            nc.sync.dma_start(out=w_a, in_=wa.rearrange("(k p) n -> p k n", p=128))
            w_b = const.tile([128, 2, 256], f16)
            nc.sync.dma_start(out=w_b, in_=wb.rearrange("(k p) n -> p k n", p=128))
            w_d1 = const.tile([128, 2, 256], f16)
            nc.sync.dma_start(out=w_d1, in_=wd1.rearrange("(k p) n -> p k n", p=128))
            w_d2 = const.tile([128, 2, 2], f16)
            nc.sync.dma_start(out=w_d2, in_=wd2.rearrange("(k p) n -> p k n", p=128))

            b_a = const.tile([128, 2], f32)
            nc.sync.dma_start(out=b_a, in_=ba_in.rearrange("(m p) -> p m", p=128))
            b_b = const.tile([128, 2], f32)
            nc.sync.dma_start(out=b_b, in_=bb_in.rearrange("(m p) -> p m", p=128))
            b_d1 = const.tile([128, 2], f32)
            nc.sync.dma_start(out=b_d1, in_=bd1_in.rearrange("(m p) -> p m", p=128))

            ident = const.tile([128, 128], f32r)
            nc.sync.dma_start(out=ident, in_=ident_in[:])
            ident16 = const.tile([128, 128], f16)
            nc.sync.dma_start(out=ident16, in_=ident16_in[:])
            eps_t = const.tile([NROWS, 1], f32)
            nc.vector.memset(eps_t, EPS)

            rstd_T = const.tile([128, NBLK, NROWS], f32)

            # ---- prologue: per-pair LN stats for this core's 80 rows ----
            with (
                tc.tile_pool(name="pro_ps", bufs=2, space="PSUM") as pro_ps,
                tc.tile_pool(name="pro_sb", bufs=1) as pro_sb,
            ):
                mu_p_sb = pro_sb.tile([NROWS, NB], f32r, tag="mu_p")
                rstd_sb = pro_sb.tile([NROWS, NB], f32r, tag="rstd")
                invr_sb = pro_sb.tile([NROWS, NB], f32r, tag="invr")
                for nch in range(2):
                    seg = slice(nch * 320, (nch + 1) * 320)
                    psA = pro_ps.tile([NROWS, 320], f32, tag="psA")
                    nc.tensor.matmul(psA, lmu, rmu[:, seg], start=True, stop=True)
                    nc.vector.tensor_copy(out=mu_p_sb[:, seg], in_=psA)
                    psB = pro_ps.tile([NROWS, 320], f32, tag="psB")
                    nc.tensor.matmul(psB, lmsq, rmsq[:, seg], start=True, stop=False)
                    nc.tensor.matmul(psB, orbTc[:, 0, :], orbT[:, 0, seg],
                                     start=False, stop=False)
                    nc.tensor.matmul(psB, orbTc[:, 1, :], orbT[:, 1, seg],
                                     start=False, stop=True)
                    mu2 = pro_sb.tile([NROWS, 320], f32, tag="mu2")
                    nc.vector.tensor_mul(mu2, mu_p_sb[:, seg], mu_p_sb[:, seg])
                    nc.vector.tensor_sub(invr_sb[:, seg], psB, mu2)
                # invr = sqrt(var + eps); rstd = 1/invr
                nc.scalar.activation(out=invr_sb, in_=invr_sb, func=AF.Sqrt,
                                     bias=eps_t[:, 0:1])
                nc.vector.reciprocal(out=rstd_sb, in_=invr_sb)
                # rstd*mu_p, and fp16 casts of both rows
                rstdmu = pro_sb.tile([NROWS, NB], f32, tag="rstdmu")
                nc.vector.tensor_mul(rstdmu, rstd_sb, mu_p_sb)
                rstd16 = pro_sb.tile([NROWS, NB], f16, tag="rstd16")
                nc.vector.tensor_copy(out=rstd16, in_=rstd_sb)
                rstdmu16 = pro_sb.tile([NROWS, NB], f16, tag="rstdmu16")
                nc.vector.tensor_copy(out=rstdmu16, in_=rstdmu)
                # assemble the combined per-row operand planes in DRAM
                nc.sync.dma_start(out=row_scratch[0, :, 0:NB], in_=rstd16)
                nc.sync.dma_start(out=row_scratch[1, :, 0:NB], in_=rstdmu16)
                ones16 = pro_sb.tile([NROWS, NB], f16, tag="ones16")
                nc.vector.memset(ones16, 1.0)
                nc.sync.dma_start(out=row_scratch[2, :, 0:NB], in_=ones16)
                nc.sync.dma_start(out=row_scratch[:, :, NB:],
                                  in_=r3rows_in.rearrange("q k n -> k q n"))
                # transposed rstd for the per-row scaled-SA products
                for jt in range(NBLK):
                    pT = pro_ps.tile([128, NROWS], f32r, tag="pT")
                    nc.tensor.transpose(
                        pT, rstd_sb[:, jt * 128:(jt + 1) * 128],
                        ident[0:NROWS, 0:NROWS])
                    nc.vector.tensor_copy(out=rstd_T[:, jt, :], in_=pT)

            # ---- main loop ----
            import contextlib
            _mstack = contextlib.ExitStack()
            aT_pool = _mstack.enter_context(
                tc.tile_pool(name="aT", bufs=GROUP // 2 + 2))
            chainx = _mstack.enter_context(
                tc.tile_pool(name="chainx", bufs=int(os.environ.get("DD_CHX", "6"))))
            px4_pool = _mstack.enter_context(
                tc.tile_pool(name="px4", bufs=int(os.environ.get("DD_PX4", "2")), space="PSUM"))
            pchain = _mstack.enter_context(
                tc.tile_pool(name="pchain", bufs=int(os.environ.get("DD_PCH", "2")), space="PSUM"))

            act_prev = [None]
            nopin = bool(int(os.environ.get("DD_NOPIN", "0")))

            def act_chain(bi):
                if act_prev[0] is not None and not nopin:
                    from concourse.tile_rust import add_dep_helper
                    add_dep_helper(bi.ins, act_prev[0].ins, sync=True,
                                   reason="pin ACT order for act-table reuse")
                act_prev[0] = bi
                return bi

            row_stage = {}            # r_loc -> (ssa tile, r3 tile)

            def stage_row(r_loc):
                if r_loc in row_stage:
                    return row_stage[r_loc]
                # rstd-scaled SA: ssa[p, jt, s] = rstd[i, jt*128+p] * SA[jt*128+p, s]
                # one tensor_scalar per block keeps the DVE 2x perf modes
                ssa = ssa_pool.tile([128, NBLK, 256], f16, tag="ssa", name="ssa")
                B = r_loc // RPB  # this row's block: only jt >= B is used
                for jt in range(B, NBLK):
                    nc.vector.tensor_scalar_mul(
                        ssa[:, jt, :], sa16[:, jt, :],
                        rstd_T[:, jt, r_loc:r_loc + 1])
                rb = prow.tile([3, NB + 256], f16, tag="rb", name="rb")
                nc.sync.dma_start(out=rb, in_=row_scratch[:, r_loc, :])
                row_stage[r_loc] = (ssa, rb)
                return row_stage[r_loc]

            def ensure_row(r_loc):
                res = stage_row(r_loc)
                for ahead in (1, 2):       # prefetch upcoming rows
                    if r_loc + ahead < NROWS:
                        stage_row(r_loc + ahead)
                return res

            def score_chunk(c):
                """scores + softmax for tiles 4c..4c+3 -> attn tile (f16)."""
                px4 = px4_pool.tile([128, 4, 256], f32, tag="px4", name="px4")
                metas = []
                for ti in range(4):
                    B, r, jt = TILES[4 * c + ti]
                    r_loc = B * RPB + r
                    ssa, rb = ensure_row(r_loc)
                    metas.append((r_loc, jt, ssa))
                # per psum bank: rank-3 stats then the rstd*SA_j adds
                for h in range(2):
                    for q in range(2):
                        r_loc, jt, ssa = metas[2 * h + q]
                        _, rb = row_stage[r_loc]
                        nc.tensor.matmul(px4[:, 2 * h + q, :],
                                         rb[:, jt * 128:jt * 128 + 128],
                                         rb[:, NB:],
                                         start=(q == 0), stop=False,
                                         skip_group_check=True)
                    m0, m1 = metas[2 * h], metas[2 * h + 1]
                    last = (h == 1)
                    if m0[0] == m1[0] and m1[1] == m0[1] + 1:
                        nc.tensor.matmul(
                            px4[:, 2 * h:2 * h + 2, :].rearrange("p a s -> p (a s)"),
                            ident16,
                            m0[2][:, m0[1]:m0[1] + 2, :].rearrange("p a s -> p (a s)"),
                            start=False, stop=last, skip_group_check=True)
                    else:
                        for q in range(2):
                            r_loc, jt, ssa = metas[2 * h + q]
                            nc.tensor.matmul(px4[:, 2 * h + q, :], ident16,
                                             ssa[:, jt, :],
                                             start=False, stop=last and q == 1,
                                             skip_group_check=True)
                ee = ee_pool.tile([128, 4, 8, 32], f16, tag="ee", name="ee")
                act_chain(nc.scalar.activation(
                    out=ee.rearrange("p a h t -> p (a h t)"),
                    in_=px4.rearrange("p a s -> p (a s)"),
                    func=AF.Exp))
                den = small.tile([128, 4, 8], f16, tag="den", name="den")
                nc.vector.reduce_sum(out=den, in_=ee, axis=AX.X)
                rden = small.tile([128, 4, 8], f16, tag="rden", name="rden")
                nc.vector.reciprocal(out=rden, in_=den)
                attn = attn_pool.tile([128, 4, 8, 32], f16, tag="attn",
                                      name="attn")
                nc.gpsimd.tensor_mul(attn, ee,
                                     rden.to_broadcast([128, 4, 8, 32]))
                return attn

            def chain_layer(x_of, w, b_tile, out_tile):
                for mt in range(2):
                    ps = pchain.tile([128, 2, 512], f32, tag="pch",
                                     name="pch")
                    for kt in range(2):
                        for qi in range(2):
                            nc.tensor.matmul(
                                ps[:, qi, :],
                                w[:, kt, mt * 128:(mt + 1) * 128],
                                x_of(qi, kt),
                                start=(kt == 0), stop=(kt == 1))
                    act_chain(nc.scalar.activation(
                        out=out_tile[:, mt, :],
                        in_=ps.rearrange("p q n -> p (q n)"), func=AF.Silu,
                        bias=b_tile[:, mt:mt + 1]))

            def chain_d2(x5, cp):
                ps6 = pchain.tile([2, 2, 512], f32, tag="pch", name="ps6")
                for kt in range(2):
                    for qi in range(2):
                        nc.tensor.matmul(ps6[:, qi, :], w_d2[:, kt, :],
                                         x5[:, kt, qi * 512:(qi + 1) * 512],
                                         start=(kt == 0), stop=(kt == 1))
                # bias bd2 is added host-side during assembly
                o6 = outp.tile([2, 2, 512], f32, tag="o6", name="o6")
                nc.vector.tensor_copy(out=o6, in_=ps6)
                nc.sync.dma_start(
                    out=out_ext[cp],
                    in_=o6.rearrange("f q n -> f (q n)"))

            def chain_cps(group):
                """Interleave 1-2 chain-pairs layer by layer: one cp's
                matmuls cover the other's silu latency."""
                xs = []
                for aT, cp in group:
                    aTr = aT.rearrange("p q (a k) f -> p q a k f", k=2)
                    x2 = chainx.tile([128, 2, 1024], f16, tag="x", name="x2")
                    chain_layer(lambda qi, kt, a=aTr: a[:, qi, :, kt, :],
                                w_a, b_a, x2)
                    xs.append(x2)
                for i in range(len(group)):
                    x4 = chainx.tile([128, 2, 1024], f16, tag="x", name="x4")
                    chain_layer(lambda qi, kt, x=xs[i]:
                                x[:, kt, qi * 512:(qi + 1) * 512],
                                w_b, b_b, x4)
                    xs[i] = x4
                for i in range(len(group)):
                    x5 = chainx.tile([128, 2, 1024], f16, tag="x", name="x5")
                    chain_layer(lambda qi, kt, x=xs[i]:
                                x[:, kt, qi * 512:(qi + 1) * 512],
                                w_d1, b_d1, x5)
                    xs[i] = x5
                for i, (aT, cp) in enumerate(group):
                    chain_d2(xs[i], cp)

            stage = int(os.environ.get("DD_STAGE", "9"))
            if stage < 2:
                dummy = outp.tile([2, 2, 512], f32, tag="o6", name="dummy")
                nc.vector.memset(dummy, 0.5)
                for q in range(n_cps):
                    nc.sync.dma_start(out=out_ext[q],
                                      in_=dummy.rearrange("f q n -> f (q n)"))
            else:
                n_super = (n_chunks + GROUP - 1) // GROUP
                pstage = int(os.environ.get("DD_PSTAGE", "14"))

                def rows_of_super(sc):
                    rows = []
                    for c in range(sc * GROUP, min((sc + 1) * GROUP, n_chunks)):
                        for ti in range(4):
                            B, r, jt = TILES[4 * c + ti]
                            r_loc = B * RPB + r
                            if r_loc not in rows:
                                rows.append(r_loc)
                    return rows

                for r in rows_of_super(0)[:pstage]:
                    stage_row(r)
                pending = []
                for sc in range(n_super):
                    qs = list(range(sc * GROUP, min((sc + 1) * GROUP, n_chunks)))
                    ready = []
                    aT = None
                    for k, c in enumerate(qs):
                        if k % 2 == 0:
                            aT = aT_pool.tile([128, 2, 8, 128], f16, tag="aT",
                                              name="aT")
                        attn = score_chunk(c)
                        nc.sync.dma_start(
                            out=aT[:, k % 2],
                            in_=attn.rearrange("p a h t -> p (a h t)"),
                            transpose=True)
                        if k % 2 == 1:
                            ready.append((aT, c // 2))
                    # chains of the previous super, interleaved with staging
                    # of the next super's rows: the DVE staging muls land in
                    # the chain window where the vector engine is idle
                    nxt = [r for r in rows_of_super(sc + 1)
                           if r not in row_stage][:pstage] if sc + 1 < n_super else []
                    groups = [pending[i:i + 2] for i in range(0, len(pending), 2)]
                    per = -(-len(nxt) // max(1, len(groups))) if groups else 0
                    for ci, grp in enumerate(groups):
                        chain_cps(grp)
                        for r in nxt[ci * per:(ci + 1) * per]:
                            stage_row(r)
                    for r in (nxt[len(groups) * per:] if groups else nxt):
                        stage_row(r)
                    pending = ready
                for i in range(0, len(pending), 2):
                    chain_cps(pending[i:i + 2])
            _mstack.close()
    nc.compile()
    return nc


def _get_nc(n_chunks):
    key = ("nc", n_chunks)
    if key not in _CACHE:
        _CACHE[key] = _build_nc(n_chunks)
    return _CACHE[key]


def kernel(**inputs):
    from concourse.bass_utils import run_bass_kernel_spmd

    n_chunks = int(os.environ.get("DD_CHUNKS", NCHUNKS))
    pc = _precompute(inputs)
    in_maps = [_core_inputs(pc, c) for c in range(NCORES)]
    nc = _get_nc(n_chunks)
    res = run_bass_kernel_spmd(nc, in_maps, core_ids=list(range(NCORES)),
                               trace=bool(int(os.environ.get("DD_TRACE", "0"))))
    _CACHE["last_result"] = res

    R = np.zeros((NB, NB, 2), np.float32)
    for c in range(NCORES):
        o = res.results[c]["out"] + pc["bd2"][None, :, None]   # (NCPS, 2, 1024)
        ot = o.reshape(NCPS, 2, CPT, 128).transpose(0, 2, 1, 3).reshape(-1, 2, 128)
        for t in range(n_chunks * CHUNK):
            B, r, jt = TILES[t]
            i = B * 128 + r * NCORES + c
            R[i, jt * 128:(jt + 1) * 128, 0] = ot[t, 0]
            R[i, jt * 128:(jt + 1) * 128, 1] = ot[t, 1]
    for bi in range(NBLK):
        for bj in range(bi):
            R[bi * 128:(bi + 1) * 128, bj * 128:(bj + 1) * 128] = \
                R[bj * 128:(bj + 1) * 128, bi * 128:(bi + 1) * 128].transpose(1, 0, 2)

    rho = (R[:, :, 0] + 1j * R[:, :, 1]).astype(np.complex64)
    n_spin = int(np.asarray(inputs["n_spin"]))
    return np.broadcast_to(rho[None], (n_spin, NB, NB)).copy()


# revision 13
# speedup vs baseline: 1.6268x; 1.0151x over previous
"""Trainium2 Bass kernel for nn_DensityDecoder (gnn_message_passing).

Math: for every ordered pair (i, j) of NB=640 orbitals,
    pair = orb_i + orb_j
    qn   = LayerNorm(pair) ; q = qn @ Wq + bq
    attn = softmax(q . k / sqrt(Dh)) over a tiny T=32 latent KV
    out  = MLP(attn @ V @ Wo)  ->  2 values -> rho[i, j] = out0 + 1j*out1

LN statistics decompose exactly over pair = orb_i + orb_j, so the whole
pre-softmax pipeline collapses to per-orbital precomputes projected into
(head, token) score space:
    scores_ij = rstd_ij * (SA_i + SA_j - mu_ij*Sw) + Sb      (pre-scaled 1/sqrt(Dh))

Per 4-tile chunk the scores land in one [128, 4, 256] PSUM tile (rank-3
stats matmuls + rstd-scaled-SA identity adds), one Exp covers the chunk,
and the attn -> attn^T reshuffle for the feature chain runs on the DMA
XBAR transpose (16x128 tiles) instead of PE transpose matmuls, writing
fp16 straight into SBUF in [s, (tile,kt), pair] block layout.  The MLP
chain runs fp16 end to end (weights + activations; PSUM accumulation is
fp32).

rho is symmetric; only j-blocks >= i-block are computed (240 of 400 tiles),
the lower triangle is mirrored host-side.

Sharding: rows i striped across 8 cores (i % 8 == core): identical SPMD
instruction stream, 80 rows -> 240 tiles of 128 pairs -> 30 chain-pairs.
"""

import os
import numpy as np

EPS = 1e-5
H = 8
D = 256
T = 32
Dh = D // H
NB = 640
NCORES = 8
NBLK = NB // 128          # 5 column blocks
RPB = 128 // NCORES       # 16 rows per block per core
NROWS = NBLK * RPB        # 80 rows per core
TILES = [(B, r, jt) for B in range(NBLK) for r in range(RPB) for jt in range(B, NBLK)]
NTILES = len(TILES)       # 240
CHUNK = 4                 # tiles per score chunk (one [128, 4, 256] psum)
CPT = 2 * CHUNK           # tiles per chain-pair (2 chunks)
GROUP = int(os.environ.get("DD_GROUP", "8"))  # chunks per superchunk
NCHUNKS = NTILES // CHUNK  # 60
NCPS = NTILES // CPT       # 30

_CACHE = {}


def _silu(x):
    return x / (1.0 + np.exp(-x))


def _ln(x, g, b):
    mu = x.mean(-1, keepdims=True)
    var = x.var(-1, keepdims=True)
    return (x - mu) / np.sqrt(var + EPS) * g + b


def _precompute(inputs):
    """Pair-independent precompute (all O(NB*D) or smaller)."""
    f = {}
    for k, v in inputs.items():
        v = np.asarray(v)
        f[k] = v.astype(np.float64) if v.dtype in (np.float32, np.float64) else v
    Z = np.asarray(inputs["Z"]).astype(np.int64)
    l = np.asarray(inputs["l"]).astype(np.int64)
    m = np.asarray(inputs["m"]).astype(np.int64)
    m_idx = np.clip(m + 3, 0, 4)
    emb = np.concatenate([f["elem_tab"][Z], f["l_tab"][l], f["m_tab"][m_idx]], -1)
    orb = _silu(emb @ f["Wp0"] + f["bp0"]) @ f["Wp1"] + f["bp1"]          # (NB, D)

    kv = _ln(f["latent"], f["ln_gkv"], f["ln_bkv"])
    k = (kv @ f["Wk"] + f["bk"]).reshape(T, H, Dh)
    v = (kv @ f["Wv"] + f["bv"]).reshape(T, H, Dh)

    g, b = f["ln_gq"], f["ln_bq"]
    mu = orb.mean(-1)
    msq = (orb * orb).mean(-1)

    A = (orb * g) @ f["Wq"]
    wbar = g @ f["Wq"]
    bq_eff = b @ f["Wq"] + f["bqa"]

    kT = k.transpose(1, 2, 0)                                            # (H, Dh, T)
    scale = 1.0 / np.sqrt(np.float64(Dh))

    def to_scores(x):
        xh = x.reshape(x.shape[:-1] + (H, Dh))
        return (np.einsum('...hd,hdt->...ht', xh, kT).reshape(x.shape[:-1] + (H * T,))
                * scale)

    SA = to_scores(A)                                                    # (NB, 256)
    Sw = to_scores(wbar)                                                 # (256,)
    Sb = to_scores(bq_eff)                                               # (256,)
    Wvo = np.einsum('thd,hde->hte', v, f["Wo"].reshape(H, Dh, D)).reshape(H * T, D)
    # fuse consecutive linear layers (no nonlinearity between them)
    Wa = Wvo @ f["Wt0"]
    ba = f["bo"] @ f["Wt0"] + f["bt0"]
    Wb = f["Wt1"] @ f["Wd0"]
    bb = f["bt1"] @ f["Wd0"] + f["bd0"]

    fl = lambda x: np.ascontiguousarray(x, np.float32)
    return {
        "SA": fl(SA), "Sw": fl(Sw), "Sb": fl(Sb), "mu": fl(mu), "msq": fl(msq),
        "orbT_s": fl(orb.T * np.sqrt(2.0 / D)),                          # (D, NB)
        "Wa": fl(Wa), "ba": fl(ba), "Wb": fl(Wb), "bb": fl(bb),
        "Wd1": fl(f["Wd1"]), "bd1": fl(f["bd1"]),
        "Wd2": fl(f["Wd2"]), "bd2": fl(f["bd2"]),
    }


def core_rows(c):
    return [B * 128 + r * NCORES + c for B in range(NBLK) for r in range(RPB)]


def _core_inputs(pc, c):
    rows = core_rows(c)
    f16 = np.float16
    # per local row r the rank-3 rhs rows [SA_i; -Sw; Sb]
    r3rows = np.zeros((NROWS, 3, 256), f16)
    for r, i in enumerate(rows):
        r3rows[r, 0] = pc["SA"][i]
        r3rows[r, 1] = -pc["Sw"]
        r3rows[r, 2] = pc["Sb"]
    ones80 = np.ones(NROWS, np.float32)
    return {
        "sa_in": pc["SA"].astype(f16),
        "r3rows_in": r3rows,
        "orbT_in": pc["orbT_s"],
        "orbTc_in": np.ascontiguousarray(pc["orbT_s"][:, rows]),
        "lhs_mu": np.ascontiguousarray(np.stack([ones80, pc["mu"][rows]])),
        "lhs_msq": np.ascontiguousarray(np.stack([ones80, pc["msq"][rows]])),
        "rhs_mu": np.ascontiguousarray(np.stack([pc["mu"], np.ones(NB, np.float32)])),
        "rhs_msq": np.ascontiguousarray(np.stack([pc["msq"], np.ones(NB, np.float32)])),
        "ident_in": np.eye(128, dtype=np.float32),
        "ident16_in": np.eye(128, dtype=f16),
        "wa": pc["Wa"].astype(f16), "wb": pc["Wb"].astype(f16),
        "wd1": pc["Wd1"].astype(f16), "wd2": pc["Wd2"].astype(f16),
        "ba_in": pc["ba"], "bb_in": pc["bb"], "bd1_in": pc["bd1"],
    }


def _build_nc(n_chunks):
    import concourse.bass as bass
    import concourse.bacc as bacc
    import concourse.tile as tile
    from concourse import mybir
    dt = mybir.dt
    f32 = dt.float32
    f32r = dt.float32r
    f16 = dt.float16
    AF = mybir.ActivationFunctionType
    AX = mybir.AxisListType

    assert n_chunks % 2 == 0
    n_cps = n_chunks // 2

    nc = bacc.Bacc(None, target_bir_lowering=False)

    ein = lambda name, shape, d=f32: nc.dram_tensor(name, shape, d,
                                                     kind="ExternalInput")
    sa_in = ein("sa_in", [NB, 256], f16)
    r3rows_in = ein("r3rows_in", [NROWS, 3, 256], f16)
    orbT_in = ein("orbT_in", [D, NB], f32r)
    orbTc_in = ein("orbTc_in", [D, NROWS], f32r)
    lhs_mu = ein("lhs_mu", [2, NROWS], f32r)
    lhs_msq = ein("lhs_msq", [2, NROWS], f32r)
    rhs_mu = ein("rhs_mu", [2, NB], f32r)
    rhs_msq = ein("rhs_msq", [2, NB], f32r)
    ident_in = ein("ident_in", [128, 128], f32r)
    ident16_in = ein("ident16_in", [128, 128], f16)
    wa = ein("wa", [256, 256], f16)
    wb = ein("wb", [256, 256], f16)
    wd1 = ein("wd1", [256, 256], f16)
    wd2 = ein("wd2", [256, 2], f16)
    ba_in = ein("ba_in", [256])
    bb_in = ein("bb_in", [256])
    bd1_in = ein("bd1_in", [256])

    out_ext = nc.dram_tensor("out", [NCPS, 2, 1024], f32, kind="ExternalOutput")
    # combined per-row operands: [rstd | SA_i; rstd*mu | -Sw; ones | Sb]
    row_scratch = nc.dram_tensor("row_scratch", [3, NROWS, NB + 256], f16)

    with tile.TileContext(nc) as tc, \
            nc.allow_low_precision(reason="fp16 pipeline by design"):
        with (
            tc.tile_pool(name="const", bufs=1) as const,
            tc.tile_pool(name="ssa", bufs=int(os.environ.get("DD_SSA", "16"))) as ssa_pool,
            tc.tile_pool(name="prow", bufs=int(os.environ.get("DD_PROW", "16"))) as prow,
            tc.tile_pool(name="ee", bufs=int(os.environ.get("DD_EE", "8"))) as ee_pool,
            tc.tile_pool(name="attnp", bufs=int(os.environ.get("DD_ATTN", "10"))) as attn_pool,
            tc.tile_pool(name="small", bufs=4) as small,
            tc.tile_pool(name="outp", bufs=2) as outp,
        ):
            # ---- constants into SBUF (prologue-critical tensors first so
            # the prologue matmuls start as early as possible) ----
            lmu = const.tile([2, NROWS], f32r)
            nc.sync.dma_start(out=lmu, in_=lhs_mu[:])
            lmsq = const.tile([2, NROWS], f32r)
            nc.sync.dma_start(out=lmsq, in_=lhs_msq[:])
            rmu = const.tile([2, NB], f32r)
            nc.sync.dma_start(out=rmu, in_=rhs_mu[:])
            rmsq = const.tile([2, NB], f32r)
            nc.sync.dma_start(out=rmsq, in_=rhs_msq[:])
            orbT = const.tile([128, 2, NB], f32r)
            nc.sync.dma_start(out=orbT, in_=orbT_in.rearrange("(k p) n -> p k n", p=128))
            orbTc = const.tile([128, 2, NROWS], f32r)
            nc.sync.dma_start(out=orbTc, in_=orbTc_in.rearrange("(k p) m -> p k m", p=128))
            ident = const.tile([128, 128], f32r)
            nc.sync.dma_start(out=ident, in_=ident_in[:])
            sa16 = const.tile([128, NBLK, 256], f16)
            nc.sync.dma_start(out=sa16, in_=sa_in.rearrange("(jt p) c -> p jt c", p=128))

            w_a = const.tile([128, 2, 256], f16)
            nc.sync.dma_start(out=w_a, in_=wa.rearrange("(k p) n -> p k n", p=128))
            w_b = const.tile([128, 2, 256], f16)
            nc.sync.dma_start(out=w_b, in_=wb.rearrange("(k p) n -> p k n", p=128))
            w_d1 = const.tile([128, 2, 256], f16)
            nc.sync.dma_start(out=w_d1, in_=wd1.rearrange("(k p) n -> p k n", p=128))
            w_d2 = const.tile([128, 2, 2], f16)
            nc.sync.dma_start(out=w_d2, in_=wd2.rearrange("(k p) n -> p k n", p=128))

            b_a = const.tile([128, 2], f32)
            nc.sync.dma_start(out=b_a, in_=ba_in.rearrange("(m p) -> p m", p=128))
            b_b = const.tile([128, 2], f32)
            nc.sync.dma_start(out=b_b, in_=bb_in.rearrange("(m p) -> p m", p=128))
            b_d1 = const.tile([128, 2], f32)
            nc.sync.dma_start(out=b_d1, in_=bd1_in.rearrange("(m p) -> p m", p=128))

            ident16 = const.tile([128, 128], f16)
            nc.sync.dma_start(out=ident16, in_=ident16_in[:])
            eps_t = const.tile([NROWS, 1], f32)
            nc.vector.memset(eps_t, EPS)

            rstd_T = const.tile([128, NBLK, NROWS], f32)

            # ---- prologue: per-pair LN stats for this core's 80 rows ----
            with (
                tc.tile_pool(name="pro_ps", bufs=2, space="PSUM") as pro_ps,
                tc.tile_pool(name="pro_sb", bufs=1) as pro_sb,
            ):
                mu_p_sb = pro_sb.tile([NROWS, NB], f32r, tag="mu_p")
                rstd_sb = pro_sb.tile([NROWS, NB], f32r, tag="rstd")
                invr_sb = pro_sb.tile([NROWS, NB], f32r, tag="invr")
                for nch in range(2):
                    seg = slice(nch * 320, (nch + 1) * 320)
                    psA = pro_ps.tile([NROWS, 320], f32, tag="psA")
                    nc.tensor.matmul(psA, lmu, rmu[:, seg], start=True, stop=True)
                    nc.vector.tensor_copy(out=mu_p_sb[:, seg], in_=psA)
                    psB = pro_ps.tile([NROWS, 320], f32, tag="psB")
                    nc.tensor.matmul(psB, lmsq, rmsq[:, seg], start=True, stop=False)
                    nc.tensor.matmul(psB, orbTc[:, 0, :], orbT[:, 0, seg],
                                     start=False, stop=False)
                    nc.tensor.matmul(psB, orbTc[:, 1, :], orbT[:, 1, seg],
                                     start=False, stop=True)
                    mu2 = pro_sb.tile([NROWS, 320], f32, tag="mu2")
                    nc.vector.tensor_mul(mu2, mu_p_sb[:, seg], mu_p_sb[:, seg])
                    nc.vector.tensor_sub(invr_sb[:, seg], psB, mu2)
                # invr = sqrt(var + eps); rstd = 1/invr
                nc.scalar.activation(out=invr_sb, in_=invr_sb, func=AF.Sqrt,
                                     bias=eps_t[:, 0:1])
                nc.vector.reciprocal(out=rstd_sb, in_=invr_sb)
                # rstd*mu_p, and fp16 casts of both rows
                rstdmu = pro_sb.tile([NROWS, NB], f32, tag="rstdmu")
                nc.vector.tensor_mul(rstdmu, rstd_sb, mu_p_sb)
                rstd16 = pro_sb.tile([NROWS, NB], f16, tag="rstd16")
                nc.vector.tensor_copy(out=rstd16, in_=rstd_sb)
                rstdmu16 = pro_sb.tile([NROWS, NB], f16, tag="rstdmu16")
                nc.vector.tensor_copy(out=rstdmu16, in_=rstdmu)
                # assemble the combined per-row operand planes in DRAM
                nc.sync.dma_start(out=row_scratch[0, :, 0:NB], in_=rstd16)
                nc.sync.dma_start(out=row_scratch[1, :, 0:NB], in_=rstdmu16)
                ones16 = pro_sb.tile([NROWS, NB], f16, tag="ones16")
                nc.vector.memset(ones16, 1.0)
                nc.sync.dma_start(out=row_scratch[2, :, 0:NB], in_=ones16)
                nc.sync.dma_start(out=row_scratch[:, :, NB:],
                                  in_=r3rows_in.rearrange("q k n -> k q n"))
                # transposed rstd for the per-row scaled-SA products
                for jt in range(NBLK):
                    pT = pro_ps.tile([128, NROWS], f32r, tag="pT")
                    nc.tensor.transpose(
                        pT, rstd_sb[:, jt * 128:(jt + 1) * 128],
                        ident[0:NROWS, 0:NROWS])
                    nc.vector.tensor_copy(out=rstd_T[:, jt, :], in_=pT)

            # ---- main loop ----
            import contextlib
            _mstack = contextlib.ExitStack()
            aT_pool = _mstack.enter_context(
                tc.tile_pool(name="aT", bufs=GROUP // 2 + 2))
            chainx = _mstack.enter_context(
                tc.tile_pool(name="chainx", bufs=int(os.environ.get("DD_CHX", "6"))))
            px4_pool = _mstack.enter_context(
                tc.tile_pool(name="px4", bufs=int(os.environ.get("DD_PX4", "2")), space="PSUM"))
            pchain = _mstack.enter_context(
                tc.tile_pool(name="pchain", bufs=int(os.environ.get("DD_PCH", "2")), space="PSUM"))

            act_prev = [None]
            nopin = bool(int(os.environ.get("DD_NOPIN", "0")))

            def act_chain(bi):
                if act_prev[0] is not None and not nopin:
                    from concourse.tile_rust import add_dep_helper
                    add_dep_helper(bi.ins, act_prev[0].ins, sync=True,
                                   reason="pin ACT order for act-table reuse")
                act_prev[0] = bi
                return bi

            row_stage = {}            # r_loc -> (ssa tile, r3 tile)

            def stage_row(r_loc):
                if r_loc in row_stage:
                    return row_stage[r_loc]
                # rstd-scaled SA: ssa[p, jt, s] = rstd[i, jt*128+p] * SA[jt*128+p, s]
                # one tensor_scalar per block keeps the DVE 2x perf modes
                ssa = ssa_pool.tile([128, NBLK, 256], f16, tag="ssa", name="ssa")
                B = r_loc // RPB  # this row's block: only jt >= B is used
                for jt in range(B, NBLK):
                    nc.vector.tensor_scalar_mul(
                        ssa[:, jt, :], sa16[:, jt, :],
                        rstd_T[:, jt, r_loc:r_loc + 1])
                rb = prow.tile([3, NB + 256], f16, tag="rb", name="rb")
                nc.sync.dma_start(out=rb, in_=row_scratch[:, r_loc, :])
                row_stage[r_loc] = (ssa, rb)
                return row_stage[r_loc]

            def ensure_row(r_loc):
                res = stage_row(r_loc)
                for ahead in (1, 2):       # prefetch upcoming rows
                    if r_loc + ahead < NROWS:
                        stage_row(r_loc + ahead)
                return res

            def score_chunk(c):
                """scores + softmax for tiles 4c..4c+3 -> attn tile (f16)."""
                px4 = px4_pool.tile([128, 4, 256], f32, tag="px4", name="px4")
                metas = []
                for ti in range(4):
                    B, r, jt = TILES[4 * c + ti]
                    r_loc = B * RPB + r
                    ssa, rb = ensure_row(r_loc)
                    metas.append((r_loc, jt, ssa))
                # all rank-3 stats matmuls first (they only need the rb DMA),
                # then the rstd*SA_j adds (which wait on the DVE ssa staging)
                for h in range(2):
                    for q in range(2):
                        r_loc, jt, ssa = metas[2 * h + q]
                        _, rb = row_stage[r_loc]
                        nc.tensor.matmul(px4[:, 2 * h + q, :],
                                         rb[:, jt * 128:jt * 128 + 128],
                                         rb[:, NB:],
                                         start=(q == 0), stop=False,
                                         skip_group_check=True)
                for h in range(2):
                    m0, m1 = metas[2 * h], metas[2 * h + 1]
                    last = (h == 1)
                    if m0[0] == m1[0] and m1[1] == m0[1] + 1:
                        nc.tensor.matmul(
                            px4[:, 2 * h:2 * h + 2, :].rearrange("p a s -> p (a s)"),
                            ident16,
                            m0[2][:, m0[1]:m0[1] + 2, :].rearrange("p a s -> p (a s)"),
                            start=False, stop=last, skip_group_check=True)
                    else:
                        for q in range(2):
                            r_loc, jt, ssa = metas[2 * h + q]
                            nc.tensor.matmul(px4[:, 2 * h + q, :], ident16,
                                             ssa[:, jt, :],
                                             start=False, stop=last and q == 1,
                                             skip_group_check=True)
                ee = ee_pool.tile([128, 4, 8, 32], f16, tag="ee", name="ee")
                act_chain(nc.scalar.activation(
                    out=ee.rearrange("p a h t -> p (a h t)"),
                    in_=px4.rearrange("p a s -> p (a s)"),
                    func=AF.Exp))
                den = small.tile([128, 4, 8], f16, tag="den", name="den")
                nc.vector.reduce_sum(out=den, in_=ee, axis=AX.X)
                rden = small.tile([128, 4, 8], f16, tag="rden", name="rden")
                nc.vector.reciprocal(out=rden, in_=den)
                attn = attn_pool.tile([128, 4, 8, 32], f16, tag="attn",
                                      name="attn")
                nc.gpsimd.tensor_mul(attn, ee,
                                     rden.to_broadcast([128, 4, 8, 32]))
                return attn

            def chain_layer(x_of, w, b_tile, out_tile):
                for mt in range(2):
                    ps = pchain.tile([128, 2, 512], f32, tag="pch",
                                     name="pch")
                    for kt in range(2):
                        for qi in range(2):
                            nc.tensor.matmul(
                                ps[:, qi, :],
                                w[:, kt, mt * 128:(mt + 1) * 128],
                                x_of(qi, kt),
                                start=(kt == 0), stop=(kt == 1))
                    act_chain(nc.scalar.activation(
                        out=out_tile[:, mt, :],
                        in_=ps.rearrange("p q n -> p (q n)"), func=AF.Silu,
                        bias=b_tile[:, mt:mt + 1]))

            def chain_d2(x5, cp):
                ps6 = pchain.tile([2, 2, 512], f32, tag="pch", name="ps6")
                for kt in range(2):
                    for qi in range(2):
                        nc.tensor.matmul(ps6[:, qi, :], w_d2[:, kt, :],
                                         x5[:, kt, qi * 512:(qi + 1) * 512],
                                         start=(kt == 0), stop=(kt == 1))
                # bias bd2 is added host-side during assembly
                o6 = outp.tile([2, 2, 512], f32, tag="o6", name="o6")
                nc.vector.tensor_copy(out=o6, in_=ps6)
                nc.sync.dma_start(
                    out=out_ext[cp],
                    in_=o6.rearrange("f q n -> f (q n)"))

            def chain_cps(group):
                """Interleave 1-2 chain-pairs layer by layer: one cp's
                matmuls cover the other's silu latency."""
                xs = []
                for aT, cp in group:
                    aTr = aT.rearrange("p q (a k) f -> p q a k f", k=2)
                    x2 = chainx.tile([128, 2, 1024], f16, tag="x", name="x2")
                    chain_layer(lambda qi, kt, a=aTr: a[:, qi, :, kt, :],
                                w_a, b_a, x2)
                    xs.append(x2)
                for i in range(len(group)):
                    x4 = chainx.tile([128, 2, 1024], f16, tag="x", name="x4")
                    chain_layer(lambda qi, kt, x=xs[i]:
                                x[:, kt, qi * 512:(qi + 1) * 512],
                                w_b, b_b, x4)
                    xs[i] = x4
                for i in range(len(group)):
                    x5 = chainx.tile([128, 2, 1024], f16, tag="x", name="x5")
                    chain_layer(lambda qi, kt, x=xs[i]:
                                x[:, kt, qi * 512:(qi + 1) * 512],
                                w_d1, b_d1, x5)
                    xs[i] = x5
                for i, (aT, cp) in enumerate(group):
                    chain_d2(xs[i], cp)

            stage = int(os.environ.get("DD_STAGE", "9"))
            if stage < 2:
                dummy = outp.tile([2, 2, 512], f32, tag="o6", name="dummy")
                nc.vector.memset(dummy, 0.5)
                for q in range(n_cps):
                    nc.sync.dma_start(out=out_ext[q],
                                      in_=dummy.rearrange("f q n -> f (q n)"))
            else:
                n_super = (n_chunks + GROUP - 1) // GROUP
                pstage = int(os.environ.get("DD_PSTAGE", "14"))

                def rows_of_super(sc):
                    rows = []
                    for c in range(sc * GROUP, min((sc + 1) * GROUP, n_chunks)):
                        for ti in range(4):
                            B, r, jt = TILES[4 * c + ti]
                            r_loc = B * RPB + r
                            if r_loc not in rows:
                                rows.append(r_loc)
                    return rows

                for r in rows_of_super(0)[:pstage]:
                    stage_row(r)
                pending = []
                for sc in range(n_super):
                    qs = list(range(sc * GROUP, min((sc + 1) * GROUP, n_chunks)))
                    ready = []
                    aT = None
                    for k, c in enumerate(qs):
                        if k % 2 == 0:
                            aT = aT_pool.tile([128, 2, 8, 128], f16, tag="aT",
                                              name="aT")
                        attn = score_chunk(c)
                        nc.sync.dma_start(
                            out=aT[:, k % 2],
                            in_=attn.rearrange("p a h t -> p (a h t)"),
                            transpose=True)
                        if k % 2 == 1:
                            ready.append((aT, c // 2))
                    # chains of the previous super, interleaved with staging
                    # of the next super's rows: the DVE staging muls land in
                    # the chain window where the vector engine is idle
                    nxt = [r for r in rows_of_super(sc + 1)
                           if r not in row_stage][:pstage] if sc + 1 < n_super else []
                    groups = [pending[i:i + 2] for i in range(0, len(pending), 2)]
                    per = -(-len(nxt) // max(1, len(groups))) if groups else 0
                    for ci, grp in enumerate(groups):
                        chain_cps(grp)
                        for r in nxt[ci * per:(ci + 1) * per]:
                            stage_row(r)
                    for r in (nxt[len(groups) * per:] if groups else nxt):
                        stage_row(r)
                    pending = ready
                for i in range(0, len(pending), 2):
                    chain_cps(pending[i:i + 2])
            _mstack.close()
    nc.compile()
    return nc


def _get_nc(n_chunks):
    key = ("nc", n_chunks)
    if key not in _CACHE:
        _CACHE[key] = _build_nc(n_chunks)
    return _CACHE[key]


def kernel(**inputs):
    from concourse.bass_utils import run_bass_kernel_spmd

    n_chunks = int(os.environ.get("DD_CHUNKS", NCHUNKS))
    pc = _precompute(inputs)
    in_maps = [_core_inputs(pc, c) for c in range(NCORES)]
    nc = _get_nc(n_chunks)
    res = run_bass_kernel_spmd(nc, in_maps, core_ids=list(range(NCORES)),
                               trace=bool(int(os.environ.get("DD_TRACE", "0"))))
    _CACHE["last_result"] = res

    R = np.zeros((NB, NB, 2), np.float32)
    for c in range(NCORES):
        o = res.results[c]["out"] + pc["bd2"][None, :, None]   # (NCPS, 2, 1024)
        ot = o.reshape(NCPS, 2, CPT, 128).transpose(0, 2, 1, 3).reshape(-1, 2, 128)
        for t in range(n_chunks * CHUNK):
            B, r, jt = TILES[t]
            i = B * 128 + r * NCORES + c
            R[i, jt * 128:(jt + 1) * 128, 0] = ot[t, 0]
            R[i, jt * 128:(jt + 1) * 128, 1] = ot[t, 1]
    for bi in range(NBLK):
        for bj in range(bi):
            R[bi * 128:(bi + 1) * 128, bj * 128:(bj + 1) * 128] = \
                R[bj * 128:(bj + 1) * 128, bi * 128:(bi + 1) * 128].transpose(1, 0, 2)

    rho = (R[:, :, 0] + 1j * R[:, :, 1]).astype(np.complex64)
    n_spin = int(np.asarray(inputs["n_spin"]))
    return np.broadcast_to(rho[None], (n_spin, NB, NB)).copy()
